# revision 1
# baseline (speedup 1.0000x reference)
"""ContrastivePretrainedSAGE Trainium2 kernel v2 (8-core SPMD).

Design: nodes sharded by id range (12544 slots/core = 98 windows of 128).
Edges routed to the dst-owning core, ordered (superwindow, src-group,
window, tile). Source features are fetched with InstDMAGatherAnt
(`dma_gather`): bf16 x rows (512B), 1024 rows per op, two SWDGE queues so
descriptor generation overlaps the previous op's transfer. Segment-sum via
one-hot mask matmuls (mask built on DVE from a slot table), accumulating a
[128,256] f32 PSUM tile per window across the window's 20 tiles (4 groups x
5). Epilogue per window fuses the whole model:
  aggr=(psum)*inv(max(deg,1)); h=relu(aggr@W_l.T+b_l+x@W_r.T);
  gnn=h.w_score + x@(W_res.T@w_score) + (b_res.w_score+b_score);
  out=sig(a)*rer+(1-sig(a))*gnn
with deg precomputed host-side (index-space bincount) and x@[W_r.T|u]
matmuls fed from a resident bf16 x^T slice of the core's own nodes.
"""
import math
from dataclasses import dataclass

import numpy as np
import ml_dtypes

import concourse.bass as bass
import concourse.mybir as mybir
import concourse.tile as tile
from concourse.bass_utils import run_bass_kernel_spmd

F32 = mybir.dt.float32
BF16 = mybir.dt.bfloat16
I16 = mybir.dt.int16
AOP = mybir.AluOpType
ACT = mybir.ActivationFunctionType
NCORE = 8
PAD_SLOT = 255.0


def split_sync_waits(nc) -> int:
    n_split = 0
    for f in nc.m.functions:
        for bb in f.blocks:
            out = []
            changed = False
            for ins in bb.instructions:
                si = ins.sync_info
                waits = list(si.on_wait) if si is not None and si.on_wait else []
                if len(waits) > 1:
                    for g, w in enumerate(waits[:-1]):
                        nop = mybir.InstNoOp(name=f"{ins.name}-waitsplit-{g}")
                        nop.engine = ins.engine
                        nop.sync_info = mybir.SyncInfo(on_wait=[w], on_update=[])
                        out.append(nop)
                    si.on_wait = waits[-1:]
                    changed = True
                    n_split += 1
                out.append(ins)
            if changed:
                bb.instructions.clear()
                for i in out:
                    bb.instructions.append(i)
    return n_split


def finish(nc):
    split_sync_waits(nc)
    import bass_rust
    from concourse.library_config import all_libraries, standard
    m = {}
    for lib in all_libraries:
        for it in lib.instructions:
            m[it] = m.get(it, 0) | (1 << lib.index)
    bass_rust.insert_library_loads(nc, m, len(all_libraries), standard.index)
    mybir.codegen_inst_isa_subclasses(nc)
    return nc


@dataclass
class Cfg:
    nsw: int          # superwindows per core
    bw: int           # windows per superwindow
    kt: int           # tiles per (window, group) run
    nx: int           # padded gather-table rows
    gs: int           # group size (rows per source group, <= 32768)
    ngroups: int = 4
    d_in: int = 256
    d_h: int = 128

    @property
    def wpc(self):
        return self.nsw * self.bw

    @property
    def npc(self):
        return self.wpc * 128

    @property
    def run(self):           # padded rows per (window, group)
        return self.kt * 128

    @property
    def chunk(self):         # rows per (superwindow, group)
        return self.bw * self.run

    @property
    def rows(self):          # gathered rows per core
        return self.wpc * self.ngroups * self.run

    @property
    def ntiles(self):
        return self.rows // 128

    @property
    def ops(self):           # op sizes per (sw, g) chunk
        sizes = []
        left = self.chunk
        while left > 0:
            s = min(1024, left)
            sizes.append(s)
            left -= s
        return sizes


def wrap_idx(idx: np.ndarray) -> np.ndarray:
    """[L] -> [128, L/16] int16 wrapped (i at [i%16, i//16]), replicated 8x."""
    L = len(idx)
    assert L % 16 == 0
    block = np.zeros((16, L // 16), np.int16)
    block[np.arange(L) % 16, np.arange(L) // 16] = idx.astype(np.int16)
    return np.tile(block, (8, 1))


def preprocess(x, edge_index, reranker_scores, cfg: Cfg):
    """Index-space edge routing + pure layout prep of per-core inputs."""
    N = x.shape[0]
    src = np.asarray(edge_index[0], dtype=np.int64)
    dst = np.asarray(edge_index[1], dtype=np.int64)
    xf = np.asarray(x, dtype=np.float32)
    rer = np.asarray(reranker_scores, dtype=np.float32)

    x_pad = np.zeros((cfg.nx, cfg.d_in), np.float32)
    x_pad[:N] = xf
    x_bf = x_pad.astype(ml_dtypes.bfloat16)
    xT_bf = np.ascontiguousarray(x_bf.T)          # [256, nx]

    npc, wpc, run = cfg.npc, cfg.wpc, cfg.run
    g_of = src // cfg.gs

    idx_arr = np.zeros((NCORE, cfg.rows), np.int64)
    slot_arr = np.full((NCORE, cfg.rows), PAD_SLOT, np.float32)
    deg_arr = np.zeros((NCORE, 128, wpc), np.float32)
    rer_arr = np.zeros((NCORE, 128, wpc), np.float32)
    for c in range(NCORE):
        lo = c * npc
        m = (dst >= lo) & (dst < lo + npc)
        s_c = src[m]
        d_c = dst[m] - lo
        g_c = g_of[m]
        w_c = d_c >> 7
        # stream position grouping key: (sw, g, w, arbitrary)
        sw_c = w_c // cfg.bw
        key = (sw_c * cfg.ngroups + g_c) * wpc + w_c
        # sort by src within each run for ascending-address DMA locality
        order = np.argsort(key * (1 << 17) + s_c, kind="stable")
        s_c, d_c, g_c, w_c, key = (a[order] for a in (s_c, d_c, g_c, w_c, key))
        cnt = np.bincount(key, minlength=cfg.nsw * cfg.ngroups * wpc)
        # count of run (w, g) is at key (w//bw*4+g)*wpc + w
        runmax = cnt.max()
        assert runmax <= run, (runmax, run)
        start = np.concatenate([[0], np.cumsum(cnt)[:-1]])
        pos_in_run = np.arange(len(s_c)) - start[key]
        # stream offset of each (sw,g,w) run:
        #  sw * (4*chunk) + g * chunk + (w - sw*bw) * run
        sw_of = w_c // cfg.bw
        base = (sw_of * cfg.ngroups + g_c) * cfg.chunk + (w_c - sw_of * cfg.bw) * run
        pos = base + pos_in_run
        idx_arr[c, pos] = s_c - g_c * cfg.gs
        slot_arr[c, pos] = (d_c & 127).astype(np.float32)
        # padding rows keep idx 0 (valid row in every group), slot PAD_SLOT
        node = lo + np.arange(npc)
        valid = node < N
        dv = np.zeros(npc, np.float32)
        dv[valid] = np.bincount(dst, minlength=N)[node[valid]]
        rv = np.zeros(npc, np.float32)
        rv[valid] = rer[node[valid]]
        deg_arr[c] = dv.reshape(wpc, 128).T
        rer_arr[c] = rv.reshape(wpc, 128).T

    # pad rows repeat the run's last (max) real idx for DMA page locality;
    # they are slot-masked, so any in-range value is correct
    idx_arr = np.maximum.accumulate(
        idx_arr.reshape(NCORE, -1, cfg.run), axis=2).reshape(NCORE, cfg.rows)

    slot_tab = slot_arr.reshape(NCORE, cfg.ntiles, 128).transpose(0, 2, 1)
    slot_tab = np.ascontiguousarray(slot_tab.astype(ml_dtypes.bfloat16))
    idx_wrapped = np.stack([wrap_idx(idx_arr[c]) for c in range(NCORE)])

    xT_own = np.zeros((NCORE, 2, 128, cfg.npc), ml_dtypes.bfloat16)
    for c in range(NCORE):
        lo = c * npc
        hi = min(lo + npc, N)
        xT_own[c, 0, :, :hi - lo] = xT_bf[0:128, lo:hi]
        xT_own[c, 1, :, :hi - lo] = xT_bf[128:256, lo:hi]
    return x_bf, idx_wrapped, slot_tab, deg_arr, rer_arr, xT_own


def build(cfg: Cfg):
    nc = bass.Bass("TRN2", target_bir_lowering=False, debug=False,
                   num_devices=NCORE, dynamic_dma_scratch_size=32768,
                   num_swdge_queues=2)
    D, H = cfg.d_in, cfg.d_h
    wpc, ntiles = cfg.wpc, cfg.ntiles
    xrows = nc.dram_tensor("xrows", [cfg.nx, D], BF16, kind="ExternalInput")
    idx = nc.dram_tensor("idx", [128, cfg.rows // 16], I16, kind="ExternalInput")
    slot = nc.dram_tensor("slot", [128, ntiles], BF16, kind="ExternalInput")
    deg = nc.dram_tensor("deg", [128, wpc], F32, kind="ExternalInput")
    rer = nc.dram_tensor("rer", [128, wpc], F32, kind="ExternalInput")
    xto = nc.dram_tensor("xto", [2, 128, cfg.npc], BF16, kind="ExternalInput")
    w_lT = nc.dram_tensor("w_lT", [D, H], F32, kind="ExternalInput")
    w_rT = nc.dram_tensor("w_rT", [D, H], F32, kind="ExternalInput")
    w_res = nc.dram_tensor("w_res", [H, D], F32, kind="ExternalInput")
    wsc_col = nc.dram_tensor("wsc_col", [H, 1], F32, kind="ExternalInput")
    bres_col = nc.dram_tensor("bres_col", [H, 1], F32, kind="ExternalInput")
    bl_bc = nc.dram_tensor("bl_bc", [128, H], F32, kind="ExternalInput")
    wsc_bc = nc.dram_tensor("wsc_bc", [128, H], F32, kind="ExternalInput")
    iota_bc = nc.dram_tensor("iota_bc", [128, 128], BF16, kind="ExternalInput")
    bscore = nc.dram_tensor("bscore", [1, 1], F32, kind="ExternalInput")
    alpha = nc.dram_tensor("alpha", [1, 1], F32, kind="ExternalInput")
    out = nc.dram_tensor("out", [128, wpc], F32, kind="ExternalOutput")

    op_sizes = cfg.ops

    with tile.TileContext(nc) as tc:
        with (
            tc.tile_pool(name="persist", bufs=1) as pp,
            tc.tile_pool(name="gpool", bufs=6) as gpool,
            tc.tile_pool(name="mpool", bufs=6) as mpool,
            tc.tile_pool(name="wsb", bufs=4) as wsb,
            tc.tile_pool(name="apsum", bufs=(cfg.bw + 1) // 2, space="PSUM") as apsum,
            tc.tile_pool(name="tpsum", bufs=2, space="PSUM") as tpsum,
            tc.tile_pool(name="hpsum", bufs=2, space="PSUM") as hpsum,
        ):
            # ---- persistent loads -------------------------------------
            # load the Q7 gather library up front, overlapping the persist
            # DMAs, so the first dma_gather doesn't stall on it
            from concourse import library_config
            nc.gpsimd.load_library(library_config.mlp)
            # split the idx load per superwindow so the first gather only
            # waits for its own chunk
            idx_t = pp.tile([128, cfg.rows // 16], I16)
            swcols = cfg.rows // 16 // cfg.nsw
            for s in range(cfg.nsw):
                nc.sync.dma_start(out=idx_t[:, s * swcols:(s + 1) * swcols],
                                  in_=idx[:, s * swcols:(s + 1) * swcols])
            slot_t = pp.tile([128, ntiles], BF16)
            nc.sync.dma_start(out=slot_t[:], in_=slot[:])
            deg_t = pp.tile([128, wpc], F32)
            nc.sync.dma_start(out=deg_t[:], in_=deg[:])
            rer_t = pp.tile([128, wpc], F32)
            nc.sync.dma_start(out=rer_t[:], in_=rer[:])
            iota_t = pp.tile([128, 128], BF16)
            nc.sync.dma_start(out=iota_t[:], in_=iota_bc[:])
            blb_t = pp.tile([128, H], F32)
            nc.sync.dma_start(out=blb_t[:], in_=bl_bc[:])
            wscb_t = pp.tile([128, H], F32)
            nc.sync.dma_start(out=wscb_t[:], in_=wsc_bc[:])
            xto_t = []
            for h in range(2):
                t = pp.tile([128, cfg.npc], BF16, tag=f"xto{h}")
                nc.sync.dma_start(out=t[:], in_=xto[h])
                xto_t.append(t)
            wsc_t = pp.tile([H, 1], F32)
            nc.sync.dma_start(out=wsc_t[:], in_=wsc_col[:])
            bres_t = pp.tile([H, 1], F32)
            nc.sync.dma_start(out=bres_t[:], in_=bres_col[:])
            bsc_t = pp.tile([1, 1], F32)
            nc.sync.dma_start(out=bsc_t[:], in_=bscore[:])
            alpha_t = pp.tile([1, 1], F32)
            nc.sync.dma_start(out=alpha_t[:], in_=alpha[:])
            ones_row = pp.tile([1, 128], F32)
            nc.vector.memset(ones_row[:], 1.0)
            out_t = pp.tile([128, wpc], F32)

            # inv degree for all windows
            degc = pp.tile([128, wpc], F32)
            nc.vector.tensor_scalar_max(out=degc[:], in0=deg_t[:], scalar1=1.0)
            invd = pp.tile([128, wpc], F32)
            nc.vector.reciprocal(out=invd[:], in_=degc[:])

            # W_l.T halves -> bf16 [128, H]
            wl_t = []
            for h in range(2):
                tf = pp.tile([128, H], F32, tag=f"wlf{h}")
                nc.sync.dma_start(out=tf[:], in_=w_lT[h * 128:(h + 1) * 128, :])
                t = pp.tile([128, H], BF16, tag=f"wl{h}")
                nc.vector.tensor_copy(out=t[:], in_=tf[:])
                wl_t.append(t)
            # [W_r.T | u] halves -> bf16 [128, H+1]
            wrx_t = []
            for h in range(2):
                tf = pp.tile([128, H], F32, tag=f"wrf{h}")
                nc.sync.dma_start(out=tf[:], in_=w_rT[h * 128:(h + 1) * 128, :])
                t = pp.tile([128, H + 1], BF16, tag=f"wrx{h}")
                nc.vector.tensor_copy(out=t[:, 0:H], in_=tf[:])
                wres_h = pp.tile([H, 128], F32, tag=f"wres{h}")
                nc.sync.dma_start(out=wres_h[:], in_=w_res[:, h * 128:(h + 1) * 128])
                pu = hpsum.tile([128, 1], F32, tag="ph", name="pu")
                nc.tensor.matmul(pu[:], lhsT=wres_h[:], rhs=wsc_t[:],
                                 start=True, stop=True)
                nc.vector.tensor_copy(out=t[:, H:H + 1], in_=pu[:])
                wrx_t.append(t)

            # c = b_res @ w_score + b_score ; a = sigmoid(alpha)
            pc = hpsum.tile([1, 1], F32, tag="ph", name="pc")
            nc.tensor.matmul(pc[:], lhsT=bres_t[:], rhs=wsc_t[:],
                             start=True, stop=True)
            c_t = pp.tile([1, 1], F32)
            nc.vector.tensor_add(out=c_t[:], in0=pc[:], in1=bsc_t[:])
            a_t = pp.tile([1, 1], F32)
            nc.scalar.activation(out=a_t[:], in_=alpha_t[:], func=ACT.Sigmoid)
            oma_t = pp.tile([1, 1], F32)
            nc.vector.tensor_scalar(out=oma_t[:], in0=a_t[:], scalar1=-1.0,
                                    scalar2=1.0, op0=AOP.mult, op1=AOP.add)
            abc_row = pp.tile([1, 3], F32)
            nc.vector.tensor_copy(out=abc_row[:, 0:1], in_=a_t[:])
            nc.vector.tensor_copy(out=abc_row[:, 1:2], in_=oma_t[:])
            nc.vector.tensor_copy(out=abc_row[:, 2:3], in_=c_t[:])
            pbc = hpsum.tile([128, 3], F32, tag="ph", name="pbc")
            nc.tensor.matmul(pbc[:], lhsT=ones_row[:], rhs=abc_row[:],
                             start=True, stop=True)
            abc_t = pp.tile([128, 3], F32)
            nc.vector.tensor_copy(out=abc_t[:], in_=pbc[:])
            a_col, oma_col, c_col = abc_t[:, 0:1], abc_t[:, 1:2], abc_t[:, 2:3]

            # make a bf16 identity for transposes
            ident = pp.tile([128, 128], BF16)
            from concourse.masks import make_identity
            make_identity(nc, ident[:])

            kregs = {}
            for s in set(op_sizes):
                kregs[s] = nc.gpsimd.to_reg(s)

            def epilogue(w, acc):
                aggr = wsb.tile([128, D], BF16, tag="aggr")
                nc.vector.tensor_tensor(
                    out=aggr[:], in0=acc,
                    in1=invd[:, w:w + 1].to_broadcast([128, D]), op=AOP.mult)
                ph = hpsum.tile([128, H + 1], F32, tag="ph")
                for h in range(2):
                    nc.tensor.matmul(
                        ph[:, 0:H + 1],
                        lhsT=xto_t[h][:, w * 128:(w + 1) * 128],
                        rhs=wrx_t[h][:], start=(h == 0), stop=False)
                for h in range(2):
                    pt = tpsum.tile([128, 128], BF16, tag="pt")
                    nc.tensor.transpose(out=pt[:], in_=aggr[:, h * 128:(h + 1) * 128],
                                        identity=ident[:])
                    aggrT = wsb.tile([128, 128], BF16, tag=f"aggrT{h}")
                    nc.vector.tensor_copy(out=aggrT[:], in_=pt[:])
                    nc.tensor.matmul(ph[:, 0:H], lhsT=aggrT[:], rhs=wl_t[h][:],
                                     start=False, stop=(h == 1))
                hpre = wsb.tile([128, H], F32, tag="hpre")
                nc.vector.tensor_add(out=hpre[:], in0=ph[:, 0:H], in1=blb_t[:])
                hrelu = wsb.tile([128, H], F32, tag="hrelu")
                nc.scalar.activation(out=hrelu[:], in_=hpre[:], func=ACT.Relu)
                hw = wsb.tile([128, H], F32, tag="hw")
                nc.vector.tensor_tensor(out=hw[:], in0=hrelu[:], in1=wscb_t[:],
                                        op=AOP.mult)
                gdot = wsb.tile([128, 1], F32, tag="gdot")
                nc.vector.reduce_sum(out=gdot[:], in_=hw[:],
                                     axis=mybir.AxisListType.X)
                g1 = wsb.tile([128, 1], F32, tag="g1")
                nc.vector.tensor_add(out=g1[:], in0=gdot[:], in1=ph[:, H:H + 1])
                g2 = wsb.tile([128, 1], F32, tag="g2")
                nc.vector.tensor_add(out=g2[:], in0=g1[:], in1=c_col)
                g3 = wsb.tile([128, 1], F32, tag="g3")
                nc.vector.tensor_tensor(out=g3[:], in0=g2[:], in1=oma_col,
                                        op=AOP.mult)
                g4 = wsb.tile([128, 1], F32, tag="g4")
                nc.vector.tensor_tensor(out=g4[:], in0=rer_t[:, w:w + 1],
                                        in1=a_col, op=AOP.mult)
                nc.vector.tensor_add(out=out_t[:, w:w + 1], in0=g3[:], in1=g4[:])

            # ---- main loop --------------------------------------------
            opq = 0
            for sw in range(cfg.nsw):
                accs = {}
                for g in range(cfg.ngroups):
                    chunk_tile0 = (sw * cfg.ngroups + g) * (cfg.chunk // 128)
                    pos = 0
                    for osz in op_sizes:
                        nt = osz // 128
                        t0 = chunk_tile0 + pos // 128
                        gb = gpool.tile([128, nt, D], BF16, tag=f"gb{nt}")
                        col0 = (chunk_tile0 * 128 + pos) // 16
                        nc.gpsimd.dma_gather(
                            out_ap=gb[:], in_ap=xrows[g * cfg.gs:(g + 1) * cfg.gs, :],
                            idxs_ap=idx_t[:, col0:col0 + osz // 16],
                            num_idxs=osz, num_idxs_reg=kregs[osz],
                            elem_size=D, queue_num=opq % 2)
                        opq += 1
                        mk = mpool.tile([128, nt, 128], BF16, tag=f"mk{nt}")
                        nc.vector.tensor_tensor(
                            out=mk[:],
                            in0=slot_t[:, t0:t0 + nt].unsqueeze(2)
                                .to_broadcast([128, nt, 128]),
                            in1=iota_t[:].unsqueeze(1).to_broadcast([128, nt, 128]),
                            op=AOP.is_equal)
                        for k in range(nt):
                            t = t0 + k
                            # tile t within chunk: local = t - chunk_tile0
                            loc = t - chunk_tile0
                            wloc = loc // cfg.kt
                            w = sw * cfg.bw + wloc
                            kk = loc % cfg.kt
                            pair, sub = wloc // 2, wloc % 2
                            if g == 0 and kk == 0 and sub == 0:
                                accs[pair] = apsum.tile(
                                    [128, 2 * D], F32, tag="acc",
                                    name=f"accp{pair}")
                            acc = accs[pair][:, sub * D:(sub + 1) * D]
                            last = (g == cfg.ngroups - 1) and (kk == cfg.kt - 1)
                            # start=True zeroes the whole PSUM bank, so only
                            # the pair's very first matmul may set it; the
                            # odd window's region is zeroed by that same
                            # bank-wide start.
                            nc.tensor.matmul(acc, lhsT=mk[:, k, :],
                                             rhs=gb[:, k, :],
                                             start=(g == 0 and kk == 0
                                                    and sub == 0),
                                             stop=last)
                            if last:
                                epilogue(w, acc)
                        pos += osz

            nc.sync.dma_start(out=out[:], in_=out_t[:])

    return finish(nc)


def kernel_impl(x, edge_index, reranker_scores, W_l, b_l, W_r, W_res, b_res,
                w_score, b_score, alpha, trace=False):
    N = int(x.shape[0])
    # 98 windows = 14 superwindows x 7; 12544 slots/core
    cfg = Cfg(nsw=14, bw=7, kt=5, nx=100096, gs=25024)
    assert cfg.npc * NCORE >= N

    x_bf, idx_w, slot_tab, deg_arr, rer_arr, xT_own = preprocess(
        x, edge_index, reranker_scores, cfg)

    common = {
        "xrows": x_bf,
        "w_lT": np.ascontiguousarray(np.asarray(W_l, np.float32).T),
        "w_rT": np.ascontiguousarray(np.asarray(W_r, np.float32).T),
        "w_res": np.asarray(W_res, np.float32),
        "wsc_col": np.asarray(w_score, np.float32).reshape(cfg.d_h, 1),
        "bres_col": np.asarray(b_res, np.float32).reshape(cfg.d_h, 1),
        "bl_bc": np.ascontiguousarray(np.broadcast_to(
            np.asarray(b_l, np.float32), (128, cfg.d_h))),
        "wsc_bc": np.ascontiguousarray(np.broadcast_to(
            np.asarray(w_score, np.float32), (128, cfg.d_h))),
        "iota_bc": np.ascontiguousarray(np.broadcast_to(
            np.arange(128, dtype=np.float32), (128, 128))).astype(
                ml_dtypes.bfloat16),
        "bscore": np.asarray(b_score, np.float32).reshape(1, 1),
        "alpha": np.asarray(alpha, np.float32).reshape(1, 1),
    }
    in_maps = []
    for c in range(NCORE):
        im = dict(common)
        im["idx"] = np.ascontiguousarray(idx_w[c])
        im["slot"] = np.ascontiguousarray(slot_tab[c])
        im["deg"] = np.ascontiguousarray(deg_arr[c])
        im["rer"] = np.ascontiguousarray(rer_arr[c])
        im["xto"] = np.ascontiguousarray(xT_own[c])
        in_maps.append(im)

    nc = build(cfg)
    res = run_bass_kernel_spmd(nc, in_maps, core_ids=list(range(NCORE)),
                               trace=trace)
    pieces = []
    for c in range(NCORE):
        oc = np.asarray(res.results[c]["out"], np.float32)  # [128, wpc]
        flat = oc.T.ravel()
        lo = c * cfg.npc
        pieces.append(flat[:max(0, min(cfg.npc, N - lo))])
    full = np.concatenate(pieces).astype(np.float32)
    return (full, res) if trace else full


def kernel(**inputs):
    out = kernel_impl(
        np.asarray(inputs["x"]),
        np.asarray(inputs["edge_index"]),
        np.asarray(inputs["reranker_scores"]),
        np.asarray(inputs["W_l"]),
        np.asarray(inputs["b_l"]),
        np.asarray(inputs["W_r"]),
        np.asarray(inputs["W_res"]),
        np.asarray(inputs["b_res"]),
        np.asarray(inputs["w_score"]),
        np.asarray(inputs["b_score"]),
        np.asarray(inputs["alpha"]),
    )
    return out.astype(np.float32)



# revision 10
# speedup vs baseline: 1.3870x; 1.3870x over previous
"""ContrastivePretrainedSAGE Trainium2 kernel v3 (8-core SPMD).

Design: nodes sharded by id range (12544 slots/core = 98 windows of 128).
Edges routed to the dst-owning core. Within each core, windows are
PERMUTED (sorted by incoming-edge count, descending) so that window-slot
j has a similar count on every core; the single SPMD program sizes each
(slot, src-group) run as max-over-cores ceil(cnt/16) 16-row units
(~218k gathered rows/core vs 250.9k for fixed 5-tile runs). Runs pack
back-to-back in the (superwindow, group) chunk stream; a 128-row tile
can hold segments of several windows, each handled by a matmul over the
partition sub-range (K<=128). Source features are fetched with
InstDMAGatherAnt from an fp8(e4m3) copy of x (256B rows), 1024 rows per
op, FOUR SWDGE queues so descriptor generation runs concurrently
(measured: 1q 8.8ns/idx, 2q 4.6, 4q 3.3-3.6 - the Q7 descriptor
generator is the kernel's bottleneck). Segment-sum via one-hot mask
matmuls: masks built on DVE (is_equal vs iota, fp8 out); full-tile pairs
of the same window fuse into one fp8 DoubleRow matmul (K=256/pass, 2x PE
rate). Accumulation in [128,256] f32 PSUM per window (2 windows/bank).
Epilogue per window fuses the whole model:
  aggr=(psum)*inv(max(deg,1)); h=relu(aggr@W_l.T+b_l+x@W_r.T);
  gnn=h.w_score + x@(W_res.T@w_score) + (b_res.w_score+b_score);
  out=sig(a)*rer+(1-sig(a))*gnn
with deg precomputed host-side (index-space bincount) and x@[W_r.T|u]
matmuls fed from a resident bf16 x^T slice of the core's own nodes
(bf16: only the mean-aggregation path runs through fp8).
"""
from dataclasses import dataclass

import numpy as np
import ml_dtypes

import concourse.bass as bass
import concourse.mybir as mybir
import concourse.tile as tile
from concourse.bass_utils import run_bass_kernel_spmd

F32 = mybir.dt.float32
BF16 = mybir.dt.bfloat16
FP8 = mybir.dt.float8e4
I16 = mybir.dt.int16
AOP = mybir.AluOpType
ACT = mybir.ActivationFunctionType
NCORE = 8
PAD_SLOT = 255.0
NP_FP8 = ml_dtypes.float8_e4m3
USE_FP8 = False
# run-length granularity in rows. 32 (not 16) because PE matmul operands
# may only start at base partition 0/32/64 - and runs are placed so none
# starts at 96 (a pad unit is inserted instead).
ALIGN = 128


def split_sync_waits(nc) -> int:
    n_split = 0
    for f in nc.m.functions:
        for bb in f.blocks:
            out = []
            changed = False
            for ins in bb.instructions:
                si = ins.sync_info
                waits = list(si.on_wait) if si is not None and si.on_wait else []
                if len(waits) > 1:
                    for g, w in enumerate(waits[:-1]):
                        nop = mybir.InstNoOp(name=f"{ins.name}-waitsplit-{g}")
                        nop.engine = ins.engine
                        nop.sync_info = mybir.SyncInfo(on_wait=[w], on_update=[])
                        out.append(nop)
                    si.on_wait = waits[-1:]
                    changed = True
                    n_split += 1
                out.append(ins)
            if changed:
                bb.instructions.clear()
                for i in out:
                    bb.instructions.append(i)
    return n_split


def finish(nc):
    split_sync_waits(nc)
    import bass_rust
    from concourse.library_config import all_libraries, standard
    m = {}
    for lib in all_libraries:
        for it in lib.instructions:
            m[it] = m.get(it, 0) | (1 << lib.index)
    bass_rust.insert_library_loads(nc, m, len(all_libraries), standard.index)
    mybir.codegen_inst_isa_subclasses(nc)
    return nc


@dataclass
class Cfg:
    nsw: int          # superwindows per core
    bw: int           # windows (slots) per superwindow
    nx: int           # padded gather-table rows
    gs: int           # group size (rows per source group, <= 32768)
    ngroups: int = 4
    d_in: int = 256
    d_h: int = 128
    max_op: int = 1024

    @property
    def wpc(self):
        return self.nsw * self.bw

    @property
    def npc(self):
        return self.wpc * 128


@dataclass
class Layout:
    """Unified (all-core) tile-stream layout, host-computed.

    segs:  per stream-tile, list of (j, lo, hi, last) partition segments
           (half-open [lo,hi) rows of the tile belonging to window-slot j;
           last=True marks window j's final matmul -> epilogue after it)
    ops:   gather ops (tile0, ntiles, group)
    ntiles: stream length in tiles
    """
    segs: list
    ops: list
    ntiles: int


def wrap_idx(idx: np.ndarray) -> np.ndarray:
    """[L] -> [128, L/16] int16 wrapped (i at [i%16, i//16]), replicated 8x."""
    L = len(idx)
    assert L % 16 == 0
    block = np.zeros((16, L // 16), np.int16)
    block[np.arange(L) % 16, np.arange(L) // 16] = idx.astype(np.int16)
    return np.tile(block, (8, 1))


def preprocess(x, edge_index, reranker_scores, cfg: Cfg):
    """Index-space edge routing + pure layout prep of per-core inputs."""
    N = x.shape[0]
    src = np.asarray(edge_index[0], dtype=np.int64)
    dst = np.asarray(edge_index[1], dtype=np.int64)
    xf = np.asarray(x, dtype=np.float32)
    rer = np.asarray(reranker_scores, dtype=np.float32)

    x_pad = np.zeros((cfg.nx, cfg.d_in), np.float32)
    x_pad[:N] = xf
    x_gt = np.ascontiguousarray(
        x_pad.astype(NP_FP8 if USE_FP8 else ml_dtypes.bfloat16))
    x_bf = x_pad.astype(ml_dtypes.bfloat16)
    xT_bf = np.ascontiguousarray(x_bf.T)          # [256, nx]

    npc, wpc, ng = cfg.npc, cfg.wpc, cfg.ngroups
    g_of = src // cfg.gs
    deg_full = np.bincount(dst, minlength=N)

    # per-core routing + window permutation (sorted by count desc)
    perm = np.zeros((NCORE, wpc), np.int64)       # slot j -> orig window
    cnts = np.zeros((NCORE, wpc, ng), np.int64)
    es, eg, ed, ej = [], [], [], []
    for c in range(NCORE):
        lo = c * npc
        m = (dst >= lo) & (dst < lo + npc)
        s_c = src[m]
        d_c = dst[m] - lo
        g_c = g_of[m]
        w_c = d_c >> 7
        wtot = np.bincount(w_c, minlength=wpc)
        order_w = np.argsort(-wtot, kind="stable")
        perm[c] = order_w
        jmap = np.zeros(wpc, np.int64)
        jmap[order_w] = np.arange(wpc)
        j_c = jmap[w_c]
        cnts[c] = np.bincount(j_c * ng + g_c, minlength=wpc * ng).reshape(wpc, ng)
        es.append(s_c); ed.append(d_c); eg.append(g_c); ej.append(j_c)

    # unified run sizes in ALIGN-row units (max over cores, >=1 unit)
    upt = 128 // ALIGN                                # units per tile
    U = np.maximum(1, (cnts.max(axis=0) + ALIGN - 1) // ALIGN)   # [wpc, ng]

    # build unified stream: (sw, g) chunks of back-to-back runs, chunk
    # padded to a whole number of tiles
    segs, ops = [], []
    run_u0 = np.zeros((wpc, ng), np.int64)   # unit offset of each run
    upos = 0
    mt = cfg.max_op // 128
    for sw in range(cfg.nsw):
        for g in range(ng):
            chunk_u0 = upos
            bounds = []                       # (j, u_start, u_end)
            for jl in range(cfg.bw):
                j = sw * cfg.bw + jl
                if upt > 1 and upos % upt == upt - 1:
                    upos += 1     # never start a run at partition 96
                run_u0[j, g] = upos
                bounds.append((j, upos, upos + int(U[j, g])))
                upos += int(U[j, g])
            # pad chunk to tile boundary
            upos = (upos + upt - 1) // upt * upt
            t0 = chunk_u0 // upt
            ct = (upos - chunk_u0) // upt
            # tile segments
            for t in range(t0, t0 + ct):
                tu0, tu1 = t * upt, (t + 1) * upt
                tsegs = []
                for j, a, b in bounds:
                    s, e = max(a, tu0), min(b, tu1)
                    if s < e:
                        islast = (g == ng - 1 and e == b)
                        tsegs.append((j, (s - tu0) * ALIGN, (e - tu0) * ALIGN,
                                      islast))
                segs.append(tsegs)
            o = 0
            while o < ct:
                nt_op = min(mt, ct - o)
                ops.append((t0 + o, nt_op, g))
                o += nt_op
    ntiles = upos // upt
    lay = Layout(segs=segs, ops=ops, ntiles=ntiles)
    rows = ntiles * 128

    # per-core idx/slot tables in the unified layout
    idx_ws, slot_tabs = [], []
    deg_arr = np.zeros((NCORE, 128, wpc), np.float32)
    rer_arr = np.zeros((NCORE, 128, wpc), np.float32)
    for c in range(NCORE):
        s_c, d_c, g_c, j_c = es[c], ed[c], eg[c], ej[c]
        key = (j_c * ng + g_c)
        order = np.argsort(key * (1 << 17) + s_c, kind="stable")
        s_c, d_c, g_c, key = (a[order] for a in (s_c, d_c, g_c, key))
        cnt = np.bincount(key, minlength=wpc * ng)
        start = np.concatenate([[0], np.cumsum(cnt)[:-1]])
        idx_arr = np.zeros(rows, np.int64)
        slot_arr = np.full(rows, PAD_SLOT, np.float32)
        for j in range(wpc):
            for g in range(ng):
                k = j * ng + g
                n = int(cnt[k])
                p0 = int(run_u0[j, g]) * ALIGN
                if n:
                    sl = slice(start[k], start[k] + n)
                    idx_arr[p0:p0 + n] = s_c[sl] - g_c[sl] * cfg.gs
                    slot_arr[p0:p0 + n] = (d_c[sl] & 127).astype(np.float32)
                    idx_arr[p0 + n:p0 + int(U[j, g]) * ALIGN] = idx_arr[p0 + n - 1]
        idx_ws.append(np.ascontiguousarray(wrap_idx(idx_arr)))
        st = slot_arr.reshape(-1, 128).T
        slot_tabs.append(np.ascontiguousarray(st.astype(ml_dtypes.bfloat16)))

        lo = c * npc
        node = lo + (perm[c][:, None] * 128 + np.arange(128)[None, :])
        valid = node < N
        dv = np.zeros((wpc, 128), np.float32)
        dv[valid] = deg_full[node[valid]]
        rv = np.zeros((wpc, 128), np.float32)
        rv[valid] = rer[node[valid]]
        deg_arr[c] = dv.T
        rer_arr[c] = rv.T

    xT_own = np.zeros((NCORE, 2, 128, cfg.npc), ml_dtypes.bfloat16)
    for c in range(NCORE):
        lo = c * npc
        cols = (lo + perm[c][:, None] * 128 + np.arange(128)[None, :]).ravel()
        np.clip(cols, 0, cfg.nx - 1, out=cols)
        xT_own[c, 0] = xT_bf[0:128, cols]
        xT_own[c, 1] = xT_bf[128:256, cols]
    return x_gt, idx_ws, slot_tabs, deg_arr, rer_arr, xT_own, perm, lay


def build(cfg: Cfg, lay: Layout):
    nc = bass.Bass("TRN2", target_bir_lowering=False, debug=False,
                   num_devices=NCORE, dynamic_dma_scratch_size=32768,
                   num_swdge_queues=4)
    D, H = cfg.d_in, cfg.d_h
    wpc, ntiles = cfg.wpc, lay.ntiles
    GDT = FP8 if USE_FP8 else BF16
    xrows = nc.dram_tensor("xrows", [cfg.nx, D], GDT, kind="ExternalInput")
    idx = nc.dram_tensor("idx", [128, ntiles * 8], I16, kind="ExternalInput")
    slot = nc.dram_tensor("slot", [128, ntiles], BF16, kind="ExternalInput")
    deg = nc.dram_tensor("deg", [128, wpc], F32, kind="ExternalInput")
    rer = nc.dram_tensor("rer", [128, wpc], F32, kind="ExternalInput")
    xto = nc.dram_tensor("xto", [2, 128, cfg.npc], BF16, kind="ExternalInput")
    w_lT = nc.dram_tensor("w_lT", [D, H], F32, kind="ExternalInput")
    w_rT = nc.dram_tensor("w_rT", [D, H], F32, kind="ExternalInput")
    w_res = nc.dram_tensor("w_res", [H, D], F32, kind="ExternalInput")
    wsc_col = nc.dram_tensor("wsc_col", [H, 1], F32, kind="ExternalInput")
    bres_col = nc.dram_tensor("bres_col", [H, 1], F32, kind="ExternalInput")
    bl_bc = nc.dram_tensor("bl_bc", [128, H], F32, kind="ExternalInput")
    wsc_bc = nc.dram_tensor("wsc_bc", [128, H], F32, kind="ExternalInput")
    iota_bc = nc.dram_tensor("iota_bc", [128, 128], BF16, kind="ExternalInput")
    bscore = nc.dram_tensor("bscore", [1, 1], F32, kind="ExternalInput")
    alpha = nc.dram_tensor("alpha", [1, 1], F32, kind="ExternalInput")
    out = nc.dram_tensor("out", [128, wpc], F32, kind="ExternalOutput")

    with tile.TileContext(nc) as tc:
        with (
            tc.tile_pool(name="persist", bufs=1) as pp,
            tc.tile_pool(name="gpool", bufs=8) as gpool,
            tc.tile_pool(name="mpool", bufs=8) as mpool,
            tc.tile_pool(name="wsb", bufs=4) as wsb,
            tc.tile_pool(name="apsum", bufs=(cfg.bw + 1) // 2, space="PSUM") as apsum,
            tc.tile_pool(name="tpsum", bufs=2, space="PSUM") as tpsum,
            tc.tile_pool(name="hpsum", bufs=2, space="PSUM") as hpsum,
        ):
            # ---- persistent loads -------------------------------------
            from concourse import library_config
            nc.gpsimd.load_library(library_config.mlp)
            idx_t = pp.tile([128, ntiles * 8], I16)
            cols = ntiles * 8
            step = ((cols + cfg.nsw - 1) // cfg.nsw + 7) // 8 * 8
            for s in range(0, cols, step):
                e = min(s + step, cols)
                nc.sync.dma_start(out=idx_t[:, s:e], in_=idx[:, s:e])
            slot_t = pp.tile([128, ntiles], BF16)
            nc.sync.dma_start(out=slot_t[:], in_=slot[:])
            deg_t = pp.tile([128, wpc], F32)
            nc.sync.dma_start(out=deg_t[:], in_=deg[:])
            rer_t = pp.tile([128, wpc], F32)
            nc.sync.dma_start(out=rer_t[:], in_=rer[:])
            iota_t = pp.tile([128, 128], BF16)
            nc.sync.dma_start(out=iota_t[:], in_=iota_bc[:])
            blb_t = pp.tile([128, H], F32)
            nc.sync.dma_start(out=blb_t[:], in_=bl_bc[:])
            wscb_t = pp.tile([128, H], F32)
            nc.sync.dma_start(out=wscb_t[:], in_=wsc_bc[:])
            xto_t = []
            for h in range(2):
                t = pp.tile([128, cfg.npc], BF16, tag=f"xto{h}")
                nc.sync.dma_start(out=t[:], in_=xto[h])
                xto_t.append(t)
            wsc_t = pp.tile([H, 1], F32)
            nc.sync.dma_start(out=wsc_t[:], in_=wsc_col[:])
            bres_t = pp.tile([H, 1], F32)
            nc.sync.dma_start(out=bres_t[:], in_=bres_col[:])
            bsc_t = pp.tile([1, 1], F32)
            nc.sync.dma_start(out=bsc_t[:], in_=bscore[:])
            alpha_t = pp.tile([1, 1], F32)
            nc.sync.dma_start(out=alpha_t[:], in_=alpha[:])
            ones_row = pp.tile([1, 128], F32)
            nc.vector.memset(ones_row[:], 1.0)
            out_t = pp.tile([128, wpc], F32)

            degc = pp.tile([128, wpc], F32)
            nc.vector.tensor_scalar_max(out=degc[:], in0=deg_t[:], scalar1=1.0)
            invd = pp.tile([128, wpc], F32)
            nc.vector.reciprocal(out=invd[:], in_=degc[:])

            wl_t = []
            for h in range(2):
                tf = pp.tile([128, H], F32, tag=f"wlf{h}")
                nc.sync.dma_start(out=tf[:], in_=w_lT[h * 128:(h + 1) * 128, :])
                t = pp.tile([128, H], BF16, tag=f"wl{h}")
                nc.vector.tensor_copy(out=t[:], in_=tf[:])
                wl_t.append(t)
            wrx_t = []
            for h in range(2):
                tf = pp.tile([128, H], F32, tag=f"wrf{h}")
                nc.sync.dma_start(out=tf[:], in_=w_rT[h * 128:(h + 1) * 128, :])
                t = pp.tile([128, H + 1], BF16, tag=f"wrx{h}")
                nc.vector.tensor_copy(out=t[:, 0:H], in_=tf[:])
                wres_h = pp.tile([H, 128], F32, tag=f"wres{h}")
                nc.sync.dma_start(out=wres_h[:], in_=w_res[:, h * 128:(h + 1) * 128])
                pu = hpsum.tile([128, 1], F32, tag="ph", name="pu")
                nc.tensor.matmul(pu[:], lhsT=wres_h[:], rhs=wsc_t[:],
                                 start=True, stop=True)
                nc.vector.tensor_copy(out=t[:, H:H + 1], in_=pu[:])
                wrx_t.append(t)

            pc = hpsum.tile([1, 1], F32, tag="ph", name="pc")
            nc.tensor.matmul(pc[:], lhsT=bres_t[:], rhs=wsc_t[:],
                             start=True, stop=True)
            c_t = pp.tile([1, 1], F32)
            nc.vector.tensor_add(out=c_t[:], in0=pc[:], in1=bsc_t[:])
            a_t = pp.tile([1, 1], F32)
            nc.scalar.activation(out=a_t[:], in_=alpha_t[:], func=ACT.Sigmoid)
            oma_t = pp.tile([1, 1], F32)
            nc.vector.tensor_scalar(out=oma_t[:], in0=a_t[:], scalar1=-1.0,
                                    scalar2=1.0, op0=AOP.mult, op1=AOP.add)
            abc_row = pp.tile([1, 3], F32)
            nc.vector.tensor_copy(out=abc_row[:, 0:1], in_=a_t[:])
            nc.vector.tensor_copy(out=abc_row[:, 1:2], in_=oma_t[:])
            nc.vector.tensor_copy(out=abc_row[:, 2:3], in_=c_t[:])
            pbc = hpsum.tile([128, 3], F32, tag="ph", name="pbc")
            nc.tensor.matmul(pbc[:], lhsT=ones_row[:], rhs=abc_row[:],
                             start=True, stop=True)
            abc_t = pp.tile([128, 3], F32)
            nc.vector.tensor_copy(out=abc_t[:], in_=pbc[:])
            a_col, oma_col, c_col = abc_t[:, 0:1], abc_t[:, 1:2], abc_t[:, 2:3]

            ident = pp.tile([128, 128], BF16)
            from concourse.masks import make_identity
            make_identity(nc, ident[:])

            kregs = {}
            for (_, nt, _) in lay.ops:
                sz = nt * 128
                if sz not in kregs:
                    kregs[sz] = nc.gpsimd.to_reg(sz)

            def epilogue(j, acc):
                aggr = wsb.tile([128, D], BF16, tag="aggr")
                nc.vector.tensor_tensor(
                    out=aggr[:], in0=acc,
                    in1=invd[:, j:j + 1].to_broadcast([128, D]), op=AOP.mult)
                ph = hpsum.tile([128, H + 1], F32, tag="ph")
                for h in range(2):
                    nc.tensor.matmul(
                        ph[:, 0:H + 1],
                        lhsT=xto_t[h][:, j * 128:(j + 1) * 128],
                        rhs=wrx_t[h][:], start=(h == 0), stop=False)
                for h in range(2):
                    pt = tpsum.tile([128, 128], BF16, tag="pt")
                    nc.tensor.transpose(out=pt[:], in_=aggr[:, h * 128:(h + 1) * 128],
                                        identity=ident[:])
                    aggrT = wsb.tile([128, 128], BF16, tag=f"aggrT{h}")
                    nc.vector.tensor_copy(out=aggrT[:], in_=pt[:])
                    nc.tensor.matmul(ph[:, 0:H], lhsT=aggrT[:], rhs=wl_t[h][:],
                                     start=False, stop=(h == 1))
                hpre = wsb.tile([128, H], F32, tag="hpre")
                nc.vector.tensor_add(out=hpre[:], in0=ph[:, 0:H], in1=blb_t[:])
                hrelu = wsb.tile([128, H], F32, tag="hrelu")
                nc.scalar.activation(out=hrelu[:], in_=hpre[:], func=ACT.Relu)
                hw = wsb.tile([128, H], F32, tag="hw")
                nc.vector.tensor_tensor(out=hw[:], in0=hrelu[:], in1=wscb_t[:],
                                        op=AOP.mult)
                gdot = wsb.tile([128, 1], F32, tag="gdot")
                nc.vector.reduce_sum(out=gdot[:], in_=hw[:],
                                     axis=mybir.AxisListType.X)
                g1 = wsb.tile([128, 1], F32, tag="g1")
                nc.vector.tensor_add(out=g1[:], in0=gdot[:], in1=ph[:, H:H + 1])
                g2 = wsb.tile([128, 1], F32, tag="g2")
                nc.vector.tensor_add(out=g2[:], in0=g1[:], in1=c_col)
                g3 = wsb.tile([128, 1], F32, tag="g3")
                nc.vector.tensor_tensor(out=g3[:], in0=g2[:], in1=oma_col,
                                        op=AOP.mult)
                g4 = wsb.tile([128, 1], F32, tag="g4")
                nc.vector.tensor_tensor(out=g4[:], in0=rer_t[:, j:j + 1],
                                        in1=a_col, op=AOP.mult)
                nc.vector.tensor_add(out=out_t[:, j:j + 1], in0=g3[:], in1=g4[:])

            # ---- main loop --------------------------------------------
            accs = {}
            started = set()
            mtop = cfg.max_op // 128
            for opi, (t0, nt, g) in enumerate(lay.ops):
                sz = nt * 128
                gbf = gpool.tile([128, mtop, D], GDT, tag="gb")
                gb = gbf[:, 0:nt, :]
                nc.gpsimd.dma_gather(
                    out_ap=gb[:], in_ap=xrows[g * cfg.gs:(g + 1) * cfg.gs, :],
                    idxs_ap=idx_t[:, t0 * 8:t0 * 8 + sz // 16],
                    num_idxs=sz, num_idxs_reg=kregs[sz],
                    elem_size=D, queue_num=opi % 4)
                mkf = mpool.tile([128, mtop, 128], GDT, tag="mk")
                mk = mkf[:, 0:nt, :]
                nc.vector.tensor_tensor(
                    out=mk[:],
                    in0=slot_t[:, t0:t0 + nt].unsqueeze(2)
                        .to_broadcast([128, nt, 128]),
                    in1=iota_t[:].unsqueeze(1).to_broadcast([128, nt, 128]),
                    op=AOP.is_equal)

                def get_acc(j):
                    sw, jl = j // cfg.bw, j % cfg.bw
                    pkey = (sw, jl // 2)
                    sub = jl % 2
                    if pkey not in accs:
                        accs[pkey] = apsum.tile([128, 2 * D], F32, tag="acc",
                                                name=f"accp{pkey[1]}")
                    st = pkey not in started
                    started.add(pkey)
                    return accs[pkey][:, sub * D:(sub + 1) * D], st

                k = 0
                while k < nt:
                    tsegs = lay.segs[t0 + k]
                    # full-tile pair fusion (DoubleRow, fp8 only)
                    if (USE_FP8 and len(tsegs) == 1 and tsegs[0][1] == 0
                            and tsegs[0][2] == 128 and k + 1 < nt):
                        nsegs = lay.segs[t0 + k + 1]
                        if (len(nsegs) == 1 and nsegs[0][0] == tsegs[0][0]
                                and nsegs[0][1] == 0 and nsegs[0][2] == 128):
                            j = tsegs[0][0]
                            acc, st = get_acc(j)
                            last2 = nsegs[0][3]
                            nc.tensor.matmul(
                                acc, lhsT=mk[:, k:k + 2, :],
                                rhs=gb[:, k:k + 2, :],
                                start=st, stop=last2,
                                perf_mode=mybir.MatmulPerfMode.DoubleRow)
                            if last2:
                                epilogue(j, acc)
                            k += 2
                            continue
                    for (j, plo, phi, islast) in tsegs:
                        acc, st = get_acc(j)
                        # PE operands: base partition 0 (any size), 32
                        # (<=32 rows), or 64 - split a 32-based segment
                        # that crosses the 64 boundary
                        if plo == 32 and phi > 64:
                            spans = [(32, 64), (64, phi)]
                        else:
                            spans = [(plo, phi)]
                        for si, (a, b) in enumerate(spans):
                            nc.tensor.matmul(
                                acc, lhsT=mk[a:b, k, :], rhs=gb[a:b, k, :],
                                start=st and si == 0,
                                stop=islast and si == len(spans) - 1)
                        if islast:
                            epilogue(j, acc)
                    k += 1

            nc.sync.dma_start(out=out[:], in_=out_t[:])

    return finish(nc)


def kernel_impl(x, edge_index, reranker_scores, W_l, b_l, W_r, W_res, b_res,
                w_score, b_score, alpha, trace=False):
    N = int(x.shape[0])
    cfg = Cfg(nsw=14, bw=7, nx=100096, gs=25024)
    assert cfg.npc * NCORE >= N

    (x_gt, idx_ws, slot_tabs, deg_arr, rer_arr, xT_own, perm,
     lay) = preprocess(x, edge_index, reranker_scores, cfg)

    common = {
        "xrows": x_gt,
        "w_lT": np.ascontiguousarray(np.asarray(W_l, np.float32).T),
        "w_rT": np.ascontiguousarray(np.asarray(W_r, np.float32).T),
        "w_res": np.asarray(W_res, np.float32),
        "wsc_col": np.asarray(w_score, np.float32).reshape(cfg.d_h, 1),
        "bres_col": np.asarray(b_res, np.float32).reshape(cfg.d_h, 1),
        "bl_bc": np.ascontiguousarray(np.broadcast_to(
            np.asarray(b_l, np.float32), (128, cfg.d_h))),
        "wsc_bc": np.ascontiguousarray(np.broadcast_to(
            np.asarray(w_score, np.float32), (128, cfg.d_h))),
        "iota_bc": np.ascontiguousarray(np.broadcast_to(
            np.arange(128, dtype=np.float32), (128, 128))).astype(
                ml_dtypes.bfloat16),
        "bscore": np.asarray(b_score, np.float32).reshape(1, 1),
        "alpha": np.asarray(alpha, np.float32).reshape(1, 1),
    }
    nc = build(cfg, lay)
    in_maps = []
    for c in range(NCORE):
        im = dict(common)
        im["idx"] = idx_ws[c]
        im["slot"] = slot_tabs[c]
        im["deg"] = np.ascontiguousarray(deg_arr[c])
        im["rer"] = np.ascontiguousarray(rer_arr[c])
        im["xto"] = np.ascontiguousarray(xT_own[c])
        in_maps.append(im)

    res = run_bass_kernel_spmd(nc, in_maps, core_ids=list(range(NCORE)),
                               trace=trace)
    full = np.zeros(N, np.float32)
    for c in range(NCORE):
        oc = np.asarray(res.results[c]["out"], np.float32)  # [128, wpc]
        lo = c * cfg.npc
        node = lo + (perm[c][:, None] * 128 + np.arange(128)[None, :])
        valid = node < N
        full[node[valid]] = oc.T[valid]
    return (full, res) if trace else full


def kernel(**inputs):
    out = kernel_impl(
        np.asarray(inputs["x"]),
        np.asarray(inputs["edge_index"]),
        np.asarray(inputs["reranker_scores"]),
        np.asarray(inputs["W_l"]),
        np.asarray(inputs["b_l"]),
        np.asarray(inputs["W_r"]),
        np.asarray(inputs["W_res"]),
        np.asarray(inputs["b_res"]),
        np.asarray(inputs["w_score"]),
        np.asarray(inputs["b_score"]),
        np.asarray(inputs["alpha"]),
    )
    return out.astype(np.float32)


# revision 25
# speedup vs baseline: 1.7973x; 1.2959x over previous
"""ContrastivePretrainedSAGE Trainium2 kernel v4 (8-core SPMD).

Design: nodes sharded by id range (12544 slots/core = 98 windows of 128).
Edges routed to the dst-owning core. Within each core, windows are
PERMUTED (sorted by incoming-edge count, descending) so that window-slot
j has a similar count on every core; the single SPMD program sizes each
(slot, src-group) run as max-over-cores ceil(cnt/16) 16-row units
(~218k gathered rows/core vs 250.9k for fixed 5-tile runs). Runs pack
back-to-back in the (superwindow, group) chunk stream with no alignment
constraints: the slot table PHASE-ENCODES the target window
(slot16 = 128*jl + dst_slot, fp16-exact, jl unique within a chunk), so a
window's mask - built by one is_equal against that window's iota band -
automatically zeroes every row belonging to other windows or padding.
Every mask matmul is then a full K=128, base-partition-0 matmul (the
only PE config that runs reliably), regardless of where runs start/end.

Source features are fetched with InstDMAGatherAnt from an fp8(e4m3) copy
of x (256B rows), 1024 rows per op, FOUR SWDGE queues so descriptor
generation runs concurrently (measured: 1q 8.8 ns/idx, 2q 4.6, 4q
3.2-3.6 - the Q7 descriptor generator is this kernel's bottleneck).
Mask matmuls accumulate into a [128,256] f32 PSUM region per window
(2 windows/bank); adjacent tile pairs of the same window fuse into one
fp8 DoubleRow matmul (K=256/pass, 2x PE rate).

Everything derivable from the small weights is folded on the host
(u=W_res.T@w_score, c=b_res.w_score+b_score, a=sigmoid(alpha), with
(1-a) pre-multiplied into w_score/u/c and a into reranker_scores), so
the per-window epilogue is only:
  DVE:  aggr = psum * invd          (1 instr)
  PE:   ph = x@[W_r.T | u'] + ones@[b_l | c'] + aggr@W_l.T  (via 2
        transposes + 5 matmuls)
  ACT:  hrelu = relu(ph[:,0:128]), 2 PSUM->SBUF transpose copies
  DVE:  out[:,j] = reduce_add(hrelu * ws', init=ph[:,128])  (1 instr)
and one final out += a*rer over all windows. deg/invd are host-side
(index-space bincount).
"""
from dataclasses import dataclass

import numpy as np
import ml_dtypes

import concourse.bass as bass
import concourse.mybir as mybir
import concourse.tile as tile
from concourse.bass_utils import run_bass_kernel_spmd

F32 = mybir.dt.float32
F16 = mybir.dt.float16
BF16 = mybir.dt.bfloat16
FP8 = mybir.dt.float8e4
I16 = mybir.dt.int16
AOP = mybir.AluOpType
ACT = mybir.ActivationFunctionType
NCORE = 8
PAD_SLOT = 1023.0
NP_FP8 = ml_dtypes.float8_e4m3
USE_FP8 = True
USE_TTR = False      # fuse h*ws + reduce + xu via tensor_tensor_reduce
USE_ACTCOPY = False  # PSUM->SBUF transpose copies on ACT instead of DVE
USE_BLC = True      # add [b_l | c] via ones-row matmul instead of DVE
ALIGN = 16          # run-length granularity in rows


def split_sync_waits(nc) -> int:
    n_split = 0
    for f in nc.m.functions:
        for bb in f.blocks:
            out = []
            changed = False
            for ins in bb.instructions:
                si = ins.sync_info
                waits = list(si.on_wait) if si is not None and si.on_wait else []
                if len(waits) > 1:
                    for g, w in enumerate(waits[:-1]):
                        nop = mybir.InstNoOp(name=f"{ins.name}-waitsplit-{g}")
                        nop.engine = ins.engine
                        nop.sync_info = mybir.SyncInfo(on_wait=[w], on_update=[])
                        out.append(nop)
                    si.on_wait = waits[-1:]
                    changed = True
                    n_split += 1
                out.append(ins)
            if changed:
                bb.instructions.clear()
                for i in out:
                    bb.instructions.append(i)
    return n_split


def finish(nc):
    split_sync_waits(nc)
    import bass_rust
    from concourse.library_config import all_libraries, standard
    m = {}
    for lib in all_libraries:
        for it in lib.instructions:
            m[it] = m.get(it, 0) | (1 << lib.index)
    bass_rust.insert_library_loads(nc, m, len(all_libraries), standard.index)
    mybir.codegen_inst_isa_subclasses(nc)
    return nc


@dataclass
class Cfg:
    nsw: int          # superwindows per core
    bw: int           # windows (slots) per superwindow
    nx: int           # padded gather-table rows
    gs: int           # group size (rows per source group, <= 32768)
    ngroups: int = 4
    d_in: int = 256
    d_h: int = 128
    max_op: int = 1024

    @property
    def wpc(self):
        return self.nsw * self.bw

    @property
    def npc(self):
        return self.wpc * 128


@dataclass
class Layout:
    """Unified (all-core) tile-stream layout, host-computed.

    ops: list of gather ops (t0, nt, g, wins) where wins is the list of
         (j, tloc, ntw, islast): window-slot j covers op-local tiles
         [tloc, tloc+ntw); islast marks the window's final op -> its
         epilogue runs after those matmuls.
    ntiles: stream length in tiles
    """
    ops: list
    ntiles: int


def wrap_idx(idx: np.ndarray) -> np.ndarray:
    """[L] -> [128, L/16] int16 wrapped (i at [i%16, i//16]), replicated 8x."""
    L = len(idx)
    assert L % 16 == 0
    block = np.zeros((16, L // 16), np.int16)
    block[np.arange(L) % 16, np.arange(L) // 16] = idx.astype(np.int16)
    return np.tile(block, (8, 1))


def preprocess(x, edge_index, reranker_scores, cfg: Cfg):
    """Index-space edge routing + pure layout prep of per-core inputs."""
    N = x.shape[0]
    src = np.asarray(edge_index[0], dtype=np.int64)
    dst = np.asarray(edge_index[1], dtype=np.int64)
    rer = np.asarray(reranker_scores, dtype=np.float32)

    x_pad = np.zeros((cfg.nx, cfg.d_in), np.float32)
    x_pad[:N] = np.asarray(x, dtype=np.float32)
    x_gt = np.ascontiguousarray(
        x_pad.astype(NP_FP8 if USE_FP8 else ml_dtypes.bfloat16))
    xT_bf = np.ascontiguousarray(x_pad.astype(ml_dtypes.bfloat16).T)

    npc, wpc, ng = cfg.npc, cfg.wpc, cfg.ngroups
    g_of = src // cfg.gs
    deg_full = np.bincount(dst, minlength=N)

    # per-core routing + window permutation (sorted by count desc)
    perm = np.zeros((NCORE, wpc), np.int64)       # slot j -> orig window
    cnts = np.zeros((NCORE, wpc, ng), np.int64)
    es, eg, ed, ej = [], [], [], []
    for c in range(NCORE):
        lo = c * npc
        m = (dst >= lo) & (dst < lo + npc)
        s_c = src[m]
        d_c = dst[m] - lo
        g_c = g_of[m]
        w_c = d_c >> 7
        wtot = np.bincount(w_c, minlength=wpc)
        order_w = np.argsort(-wtot, kind="stable")
        perm[c] = order_w
        jmap = np.zeros(wpc, np.int64)
        jmap[order_w] = np.arange(wpc)
        j_c = jmap[w_c]
        cnts[c] = np.bincount(j_c * ng + g_c, minlength=wpc * ng).reshape(wpc, ng)
        es.append(s_c); ed.append(d_c); eg.append(g_c); ej.append(j_c)

    # unified run sizes in ALIGN-row units (max over cores, >=1 unit)
    upt = 128 // ALIGN                                # units per tile
    U = np.maximum(1, (cnts.max(axis=0) + ALIGN - 1) // ALIGN)   # [wpc, ng]

    # stream layout: (sw, g) chunks of back-to-back runs, chunk padded to
    # whole tiles; gather ops of <= max_op rows; per-op window tile spans
    ops = []
    run_u0 = np.zeros((wpc, ng), np.int64)
    upos = 0
    mt = cfg.max_op // 128
    for sw in range(cfg.nsw):
        for g in range(ng):
            chunk_u0 = upos
            bounds = []                       # (j, unit_start, unit_end)
            for jl in range(cfg.bw):
                j = sw * cfg.bw + jl
                run_u0[j, g] = upos
                bounds.append((j, upos, upos + int(U[j, g])))
                upos += int(U[j, g])
            upos = (upos + upt - 1) // upt * upt
            t0 = chunk_u0 // upt
            ct = (upos - chunk_u0) // upt
            o = 0
            while o < ct:
                nt_op = min(mt, ct - o)
                ot0, ot1 = t0 + o, t0 + o + nt_op    # op tile range
                wins = []
                for (j, a, b) in bounds:
                    ta, tb = a // upt, (b + upt - 1) // upt
                    s, e = max(ta, ot0), min(tb, ot1)
                    if s < e:
                        islast = (g == ng - 1 and e == tb)
                        wins.append((j, s - ot0, e - s, islast))
                ops.append((ot0, nt_op, g, wins))
                o += nt_op
    ntiles = upos // upt
    lay = Layout(ops=ops, ntiles=ntiles)
    rows = ntiles * 128

    # per-core idx/slot tables in the unified layout
    idx_ws, slot_tabs = [], []
    invd_arr = np.zeros((NCORE, 128, wpc), np.float32)
    rer_arr = np.zeros((NCORE, 128, wpc), np.float32)
    for c in range(NCORE):
        s_c, d_c, g_c, j_c = es[c], ed[c], eg[c], ej[c]
        key = (j_c * ng + g_c)
        order = np.argsort(key * (1 << 17) + s_c, kind="stable")
        s_c, d_c, g_c, key = (a[order] for a in (s_c, d_c, g_c, key))
        cnt = np.bincount(key, minlength=wpc * ng)
        start = np.concatenate([[0], np.cumsum(cnt)[:-1]])
        idx_arr = np.zeros(rows, np.int64)
        slot_arr = np.full(rows, PAD_SLOT, np.float32)
        for j in range(wpc):
            ph = 128.0 * (j % cfg.bw)
            for g in range(ng):
                k = j * ng + g
                n = int(cnt[k])
                p0 = int(run_u0[j, g]) * ALIGN
                if n:
                    sl = slice(start[k], start[k] + n)
                    idx_arr[p0:p0 + n] = s_c[sl] - g_c[sl] * cfg.gs
                    slot_arr[p0:p0 + n] = ph + (d_c[sl] & 127)
                    idx_arr[p0 + n:p0 + int(U[j, g]) * ALIGN] = idx_arr[p0 + n - 1]
        idx_ws.append(np.ascontiguousarray(wrap_idx(idx_arr)))
        st = slot_arr.reshape(-1, 128).T
        slot_tabs.append(np.ascontiguousarray(st.astype(np.float16)))

        lo = c * npc
        node = lo + (perm[c][:, None] * 128 + np.arange(128)[None, :])
        valid = node < N
        dv = np.zeros((wpc, 128), np.float32)
        dv[valid] = deg_full[node[valid]]
        invd_arr[c] = (1.0 / np.maximum(dv, 1.0)).T
        rv = np.zeros((wpc, 128), np.float32)
        rv[valid] = rer[node[valid]]
        rer_arr[c] = rv.T

    xT_own = np.zeros((NCORE, 2, 128, cfg.npc), ml_dtypes.bfloat16)
    for c in range(NCORE):
        lo = c * npc
        cols = (lo + perm[c][:, None] * 128 + np.arange(128)[None, :]).ravel()
        np.clip(cols, 0, cfg.nx - 1, out=cols)
        xT_own[c, 0] = xT_bf[0:128, cols]
        xT_own[c, 1] = xT_bf[128:256, cols]
    return x_gt, idx_ws, slot_tabs, invd_arr, rer_arr, xT_own, perm, lay


def build(cfg: Cfg, lay: Layout):
    nc = bass.Bass("TRN2", target_bir_lowering=False, debug=False,
                   num_devices=NCORE, dynamic_dma_scratch_size=32768,
                   num_swdge_queues=4)
    D, H = cfg.d_in, cfg.d_h
    wpc, ntiles = cfg.wpc, lay.ntiles
    GDT = FP8 if USE_FP8 else BF16
    xrows = nc.dram_tensor("xrows", [cfg.nx, D], GDT, kind="ExternalInput")
    idx = nc.dram_tensor("idx", [128, ntiles * 8], I16, kind="ExternalInput")
    slot = nc.dram_tensor("slot", [128, ntiles], F16, kind="ExternalInput")
    invd = nc.dram_tensor("invd", [128, wpc], F32, kind="ExternalInput")
    rer = nc.dram_tensor("rer", [128, wpc], F32, kind="ExternalInput")
    xto = nc.dram_tensor("xto", [2, 128, cfg.npc], BF16, kind="ExternalInput")
    wl = nc.dram_tensor("wl", [2, 128, H], BF16, kind="ExternalInput")
    wrx = nc.dram_tensor("wrx", [2, 128, H + 1], BF16, kind="ExternalInput")
    blc = nc.dram_tensor("blc", [1, H + 1], F32, kind="ExternalInput")
    blb = nc.dram_tensor("blb", [128, H], F32, kind="ExternalInput")
    wscb = nc.dram_tensor("wscb", [128, H], F32, kind="ExternalInput")
    iota7 = nc.dram_tensor("iota7", [128, cfg.bw * 128], F16,
                           kind="ExternalInput")
    out = nc.dram_tensor("out", [128, wpc], F32, kind="ExternalOutput")

    with tile.TileContext(nc) as tc:
        with (
            tc.tile_pool(name="persist", bufs=1) as pp,
            tc.tile_pool(name="gpool", bufs=8) as gpool,
            tc.tile_pool(name="mpool", bufs=8) as mpool,
            tc.tile_pool(name="wsb", bufs=4) as wsb,
            tc.tile_pool(name="apsum", bufs=(cfg.bw + 1) // 2, space="PSUM") as apsum,
            tc.tile_pool(name="tpsum", bufs=2, space="PSUM") as tpsum,
            tc.tile_pool(name="hpsum", bufs=2, space="PSUM") as hpsum,
        ):
            # ---- persistent loads -------------------------------------
            from concourse import library_config
            nc.gpsimd.load_library(library_config.mlp)
            idx_t = pp.tile([128, ntiles * 8], I16)
            cols = ntiles * 8
            step = ((cols + cfg.nsw - 1) // cfg.nsw + 7) // 8 * 8
            for s in range(0, cols, step):
                e = min(s + step, cols)
                nc.sync.dma_start(out=idx_t[:, s:e], in_=idx[:, s:e])
            slot_t = pp.tile([128, ntiles], F16)
            nc.sync.dma_start(out=slot_t[:], in_=slot[:])
            invd_t = pp.tile([128, wpc], F32)
            nc.sync.dma_start(out=invd_t[:], in_=invd[:])
            rer_t = pp.tile([128, wpc], F32)
            nc.sync.dma_start(out=rer_t[:], in_=rer[:])
            iota_t = pp.tile([128, cfg.bw * 128], F16)
            nc.sync.dma_start(out=iota_t[:], in_=iota7[:])
            wscb_t = pp.tile([128, H], F32)
            nc.sync.dma_start(out=wscb_t[:], in_=wscb[:])
            xto_t = []
            for h in range(2):
                t = pp.tile([128, cfg.npc], BF16, tag=f"xto{h}")
                nc.sync.dma_start(out=t[:], in_=xto[h])
                xto_t.append(t)
            wl_t = []
            wrx_t = []
            for h in range(2):
                t = pp.tile([128, H], BF16, tag=f"wl{h}")
                nc.sync.dma_start(out=t[:], in_=wl[h])
                wl_t.append(t)
                t2 = pp.tile([128, H + 1], BF16, tag=f"wrx{h}")
                nc.sync.dma_start(out=t2[:], in_=wrx[h])
                wrx_t.append(t2)
            blc_t = pp.tile([1, H + 1], F32)
            nc.sync.dma_start(out=blc_t[:], in_=blc[:])
            blb_t = pp.tile([128, H], F32)
            nc.sync.dma_start(out=blb_t[:], in_=blb[:])
            ones_row = pp.tile([1, 128], F32)
            nc.vector.memset(ones_row[:], 1.0)
            out_t = pp.tile([128, wpc], F32)

            ident = pp.tile([128, 128], BF16)
            from concourse.masks import make_identity
            make_identity(nc, ident[:])

            kregs = {}
            for (_, nt, _, _) in lay.ops:
                sz = nt * 128
                if sz not in kregs:
                    kregs[sz] = nc.gpsimd.to_reg(sz)

            def epilogue(j, acc):
                aggr = wsb.tile([128, D], BF16, tag="aggr")
                nc.vector.tensor_tensor(
                    out=aggr[:], in0=acc,
                    in1=invd_t[:, j:j + 1].to_broadcast([128, D]), op=AOP.mult)
                ph = hpsum.tile([128, H + 1], F32, tag="ph")
                for h in range(2):
                    nc.tensor.matmul(
                        ph[:, 0:H + 1],
                        lhsT=xto_t[h][:, j * 128:(j + 1) * 128],
                        rhs=wrx_t[h][:], start=(h == 0), stop=False)
                if USE_BLC:
                    nc.tensor.matmul(ph[:, 0:H + 1], lhsT=ones_row[:],
                                     rhs=blc_t[:], start=False, stop=False)
                for h in range(2):
                    pt = tpsum.tile([128, 128], BF16, tag="pt")
                    nc.tensor.transpose(out=pt[:],
                                        in_=aggr[:, h * 128:(h + 1) * 128],
                                        identity=ident[:])
                    aggrT = wsb.tile([128, 128], BF16, tag=f"aggrT{h}")
                    if USE_ACTCOPY:
                        nc.scalar.activation(out=aggrT[:], in_=pt[:],
                                             func=ACT.Copy)
                    else:
                        nc.vector.tensor_copy(out=aggrT[:], in_=pt[:])
                    nc.tensor.matmul(ph[:, 0:H], lhsT=aggrT[:], rhs=wl_t[h][:],
                                     start=False, stop=(h == 1))
                if USE_BLC:
                    hin = ph[:, 0:H]
                else:
                    hpre = wsb.tile([128, H], F32, tag="hpre")
                    nc.vector.tensor_add(out=hpre[:], in0=ph[:, 0:H],
                                         in1=blb_t[:])
                    hin = hpre[:]
                hrelu = wsb.tile([128, H], F32, tag="hrelu")
                nc.scalar.activation(out=hrelu[:], in_=hin, func=ACT.Relu)
                hw = wsb.tile([128, H], F32, tag="hw")
                if USE_TTR:
                    nc.vector.tensor_tensor_reduce(
                        out=hw[:], in0=hrelu[:], in1=wscb_t[:], scale=1.0,
                        scalar=ph[:, H:H + 1], op0=AOP.mult, op1=AOP.add,
                        accum_out=out_t[:, j:j + 1])
                else:
                    nc.vector.tensor_tensor(out=hw[:], in0=hrelu[:],
                                            in1=wscb_t[:], op=AOP.mult)
                    gdot = wsb.tile([128, 1], F32, tag="gdot")
                    nc.vector.reduce_sum(out=gdot[:], in_=hw[:],
                                         axis=mybir.AxisListType.X)
                    nc.vector.tensor_add(out=out_t[:, j:j + 1], in0=gdot[:],
                                         in1=ph[:, H:H + 1])

            # ---- main loop --------------------------------------------
            accs = {}
            started = set()
            for opi, (t0, nt, g, wins) in enumerate(lay.ops):
                sz = nt * 128
                gbf = gpool.tile([128, cfg.max_op // 128, D], GDT, tag="gb")
                gb = gbf[:, 0:nt, :]
                nc.gpsimd.dma_gather(
                    out_ap=gb[:], in_ap=xrows[g * cfg.gs:(g + 1) * cfg.gs, :],
                    idxs_ap=idx_t[:, t0 * 8:t0 * 8 + sz // 16],
                    num_idxs=sz, num_idxs_reg=kregs[sz],
                    elem_size=D, queue_num=opi % 4)
                for (j, tloc, ntw, islast) in wins:
                    jl = j % cfg.bw
                    mkf = mpool.tile([128, cfg.max_op // 128, 128], GDT,
                                     tag="mk")
                    mk = mkf[:, 0:ntw, :]
                    nc.vector.tensor_tensor(
                        out=mk[:],
                        in0=slot_t[:, t0 + tloc:t0 + tloc + ntw].unsqueeze(2)
                            .to_broadcast([128, ntw, 128]),
                        in1=iota_t[:, jl * 128:(jl + 1) * 128].unsqueeze(1)
                            .to_broadcast([128, ntw, 128]),
                        op=AOP.is_equal)
                    sw = j // cfg.bw
                    pkey = (sw, jl // 2)
                    sub = jl % 2
                    if pkey not in accs:
                        accs[pkey] = apsum.tile([128, 2 * D], F32, tag="acc",
                                                name=f"accp{pkey[1]}")
                    acc = accs[pkey][:, sub * D:(sub + 1) * D]
                    st = pkey not in started
                    started.add(pkey)
                    i = 0
                    while i < ntw:
                        pair = USE_FP8 and i + 1 < ntw
                        lastm = (i + (2 if pair else 1) >= ntw) and islast
                        if pair:
                            nc.tensor.matmul(
                                acc, lhsT=mk[:, i:i + 2, :],
                                rhs=gb[:, tloc + i:tloc + i + 2, :],
                                start=st and i == 0, stop=lastm,
                                perf_mode=mybir.MatmulPerfMode.DoubleRow)
                            i += 2
                        else:
                            nc.tensor.matmul(
                                acc, lhsT=mk[:, i, :], rhs=gb[:, tloc + i, :],
                                start=st and i == 0, stop=lastm)
                            i += 1
                    if islast:
                        epilogue(j, acc)

            nc.vector.tensor_add(out=out_t[:], in0=out_t[:], in1=rer_t[:])
            nc.sync.dma_start(out=out[:], in_=out_t[:])

    return finish(nc)


def kernel_impl(x, edge_index, reranker_scores, W_l, b_l, W_r, W_res, b_res,
                w_score, b_score, alpha, trace=False):
    N = int(x.shape[0])
    cfg = Cfg(nsw=14, bw=7, nx=100096, gs=25024)
    assert cfg.npc * NCORE >= N

    (x_gt, idx_ws, slot_tabs, invd_arr, rer_arr, xT_own, perm,
     lay) = preprocess(x, edge_index, reranker_scores, cfg)

    # host-folded small-weight math
    W_l = np.asarray(W_l, np.float64)
    W_r = np.asarray(W_r, np.float64)
    W_res = np.asarray(W_res, np.float64)
    w_score = np.asarray(w_score, np.float64)
    a = float(1.0 / (1.0 + np.exp(-float(np.asarray(alpha)))))
    oma = 1.0 - a
    u = W_res.T @ w_score                      # [256]
    cterm = float(np.asarray(b_res, np.float64) @ w_score
                  + float(np.asarray(b_score)))
    wrx_host = np.zeros((2, 128, cfg.d_h + 1), np.float32)
    wl_host = np.zeros((2, 128, cfg.d_h), np.float32)
    for h in range(2):
        wrx_host[h, :, 0:cfg.d_h] = W_r.T[h * 128:(h + 1) * 128, :]
        wrx_host[h, :, cfg.d_h] = oma * u[h * 128:(h + 1) * 128]
        wl_host[h] = W_l.T[h * 128:(h + 1) * 128, :]
    blc_host = np.zeros((1, cfg.d_h + 1), np.float32)
    blc_host[0, 0:cfg.d_h] = np.asarray(b_l, np.float32)
    blc_host[0, cfg.d_h] = oma * cterm
    band = np.arange(cfg.bw * 128, dtype=np.float32).astype(np.float16)
    iota_host = np.ascontiguousarray(
        np.broadcast_to(band, (128, cfg.bw * 128)))

    common = {
        "xrows": x_gt,
        "wl": wl_host.astype(ml_dtypes.bfloat16),
        "wrx": wrx_host.astype(ml_dtypes.bfloat16),
        "blc": blc_host,
        "blb": np.ascontiguousarray(np.broadcast_to(
            np.asarray(b_l, np.float32), (128, cfg.d_h))),
        "wscb": np.ascontiguousarray(np.broadcast_to(
            (oma * w_score).astype(np.float32), (128, cfg.d_h))),
        "iota7": iota_host,
    }
    rer_const = 0.0 if USE_BLC else oma * cterm
    nc = build(cfg, lay)
    in_maps = []
    for c_i in range(NCORE):
        im = dict(common)
        im["idx"] = idx_ws[c_i]
        im["slot"] = slot_tabs[c_i]
        im["invd"] = np.ascontiguousarray(invd_arr[c_i])
        im["rer"] = np.ascontiguousarray(
            (rer_arr[c_i] * a + rer_const).astype(np.float32))
        im["xto"] = np.ascontiguousarray(xT_own[c_i])
        in_maps.append(im)

    res = run_bass_kernel_spmd(nc, in_maps, core_ids=list(range(NCORE)),
                               trace=trace)
    full = np.zeros(N, np.float32)
    for c_i in range(NCORE):
        oc = np.asarray(res.results[c_i]["out"], np.float32)  # [128, wpc]
        lo = c_i * cfg.npc
        node = lo + (perm[c_i][:, None] * 128 + np.arange(128)[None, :])
        valid = node < N
        full[node[valid]] = oc.T[valid]
    return (full, res) if trace else full


def kernel(**inputs):
    out = kernel_impl(
        np.asarray(inputs["x"]),
        np.asarray(inputs["edge_index"]),
        np.asarray(inputs["reranker_scores"]),
        np.asarray(inputs["W_l"]),
        np.asarray(inputs["b_l"]),
        np.asarray(inputs["W_r"]),
        np.asarray(inputs["W_res"]),
        np.asarray(inputs["b_res"]),
        np.asarray(inputs["w_score"]),
        np.asarray(inputs["b_score"]),
        np.asarray(inputs["alpha"]),
    )
    return out.astype(np.float32)


# revision 26
# speedup vs baseline: 1.8242x; 1.0149x over previous
"""ContrastivePretrainedSAGE Trainium2 kernel v4 (8-core SPMD).

Design: nodes sharded by id range (12544 slots/core = 98 windows of 128).
Edges routed to the dst-owning core. Within each core, windows are
PERMUTED (sorted by incoming-edge count, descending) so that window-slot
j has a similar count on every core; the single SPMD program sizes each
(slot, src-group) run as max-over-cores ceil(cnt/16) 16-row units
(~218k gathered rows/core vs 250.9k for fixed 5-tile runs). Runs pack
back-to-back in the (superwindow, group) chunk stream with no alignment
constraints: the slot table PHASE-ENCODES the target window
(slot16 = 128*jl + dst_slot, fp16-exact, jl unique within a chunk), so a
window's mask - built by one is_equal against that window's iota band -
automatically zeroes every row belonging to other windows or padding.
Every mask matmul is then a full K=128, base-partition-0 matmul (the
only PE config that runs reliably), regardless of where runs start/end.

Source features are fetched with InstDMAGatherAnt from an fp8(e4m3) copy
of x (256B rows), 1024 rows per op, FOUR SWDGE queues so descriptor
generation runs concurrently (measured: 1q 8.8 ns/idx, 2q 4.6, 4q
3.2-3.6 - the Q7 descriptor generator is this kernel's bottleneck).
Mask matmuls accumulate into a [128,256] f32 PSUM region per window
(2 windows/bank); adjacent tile pairs of the same window fuse into one
fp8 DoubleRow matmul (K=256/pass, 2x PE rate).

Everything derivable from the small weights is folded on the host
(u=W_res.T@w_score, c=b_res.w_score+b_score, a=sigmoid(alpha), with
(1-a) pre-multiplied into w_score/u/c and a into reranker_scores), so
the per-window epilogue is only:
  DVE:  aggr = psum * invd          (1 instr)
  PE:   ph = x@[W_r.T | u'] + ones@[b_l | c'] + aggr@W_l.T  (via 2
        transposes + 5 matmuls)
  ACT:  hrelu = relu(ph[:,0:128]), 2 PSUM->SBUF transpose copies
  DVE:  out[:,j] = reduce_add(hrelu * ws', init=ph[:,128])  (1 instr)
and one final out += a*rer over all windows. deg/invd are host-side
(index-space bincount).
"""
from dataclasses import dataclass

import numpy as np
import ml_dtypes

import concourse.bass as bass
import concourse.mybir as mybir
import concourse.tile as tile
from concourse.bass_utils import run_bass_kernel_spmd

F32 = mybir.dt.float32
F16 = mybir.dt.float16
BF16 = mybir.dt.bfloat16
FP8 = mybir.dt.float8e4
I16 = mybir.dt.int16
AOP = mybir.AluOpType
ACT = mybir.ActivationFunctionType
NCORE = 8
PAD_SLOT = 1023.0
NP_FP8 = ml_dtypes.float8_e4m3
USE_FP8 = True
USE_TTR = False      # fuse h*ws + reduce + xu via tensor_tensor_reduce
USE_ACTCOPY = False  # PSUM->SBUF transpose copies on ACT instead of DVE
USE_BLC = True      # add [b_l | c] via ones-row matmul instead of DVE
ALIGN = 16          # run-length granularity in rows


def split_sync_waits(nc) -> int:
    n_split = 0
    for f in nc.m.functions:
        for bb in f.blocks:
            out = []
            changed = False
            for ins in bb.instructions:
                si = ins.sync_info
                waits = list(si.on_wait) if si is not None and si.on_wait else []
                if len(waits) > 1:
                    for g, w in enumerate(waits[:-1]):
                        nop = mybir.InstNoOp(name=f"{ins.name}-waitsplit-{g}")
                        nop.engine = ins.engine
                        nop.sync_info = mybir.SyncInfo(on_wait=[w], on_update=[])
                        out.append(nop)
                    si.on_wait = waits[-1:]
                    changed = True
                    n_split += 1
                out.append(ins)
            if changed:
                bb.instructions.clear()
                for i in out:
                    bb.instructions.append(i)
    return n_split


def finish(nc):
    split_sync_waits(nc)
    import bass_rust
    from concourse.library_config import all_libraries, standard
    m = {}
    for lib in all_libraries:
        for it in lib.instructions:
            m[it] = m.get(it, 0) | (1 << lib.index)
    bass_rust.insert_library_loads(nc, m, len(all_libraries), standard.index)
    mybir.codegen_inst_isa_subclasses(nc)
    return nc


@dataclass
class Cfg:
    nsw: int          # superwindows per core
    bw: int           # windows (slots) per superwindow
    nx: int           # padded gather-table rows
    gs: int           # group size (rows per source group, <= 32768)
    ngroups: int = 4
    d_in: int = 256
    d_h: int = 128
    max_op: int = 1024

    @property
    def wpc(self):
        return self.nsw * self.bw

    @property
    def npc(self):
        return self.wpc * 128


@dataclass
class Layout:
    """Unified (all-core) tile-stream layout, host-computed.

    ops: list of gather ops (t0, nt, g, wins) where wins is the list of
         (j, tloc, ntw, islast): window-slot j covers op-local tiles
         [tloc, tloc+ntw); islast marks the window's final op -> its
         epilogue runs after those matmuls.
    ntiles: stream length in tiles
    """
    ops: list
    ntiles: int


def wrap_idx(idx: np.ndarray) -> np.ndarray:
    """[L] -> [128, L/16] int16 wrapped (i at [i%16, i//16]), replicated 8x."""
    L = len(idx)
    assert L % 16 == 0
    block = np.zeros((16, L // 16), np.int16)
    block[np.arange(L) % 16, np.arange(L) // 16] = idx.astype(np.int16)
    return np.tile(block, (8, 1))


def preprocess(x, edge_index, reranker_scores, cfg: Cfg):
    """Index-space edge routing + pure layout prep of per-core inputs."""
    N = x.shape[0]
    src = np.asarray(edge_index[0], dtype=np.int64)
    dst = np.asarray(edge_index[1], dtype=np.int64)
    rer = np.asarray(reranker_scores, dtype=np.float32)

    x_pad = np.zeros((cfg.nx, cfg.d_in), np.float32)
    x_pad[:N] = np.asarray(x, dtype=np.float32)
    x_gt = np.ascontiguousarray(
        x_pad.astype(NP_FP8 if USE_FP8 else ml_dtypes.bfloat16))
    xT_bf = np.ascontiguousarray(x_pad.astype(ml_dtypes.bfloat16).T)

    npc, wpc, ng = cfg.npc, cfg.wpc, cfg.ngroups
    g_of = src // cfg.gs
    deg_full = np.bincount(dst, minlength=N)

    # per-core routing + window permutation (sorted by count desc)
    perm = np.zeros((NCORE, wpc), np.int64)       # slot j -> orig window
    cnts = np.zeros((NCORE, wpc, ng), np.int64)
    es, eg, ed, ej = [], [], [], []
    for c in range(NCORE):
        lo = c * npc
        m = (dst >= lo) & (dst < lo + npc)
        s_c = src[m]
        d_c = dst[m] - lo
        g_c = g_of[m]
        w_c = d_c >> 7
        wtot = np.bincount(w_c, minlength=wpc)
        order_w = np.argsort(-wtot, kind="stable")
        perm[c] = order_w
        jmap = np.zeros(wpc, np.int64)
        jmap[order_w] = np.arange(wpc)
        j_c = jmap[w_c]
        cnts[c] = np.bincount(j_c * ng + g_c, minlength=wpc * ng).reshape(wpc, ng)
        es.append(s_c); ed.append(d_c); eg.append(g_c); ej.append(j_c)

    # unified run sizes in ALIGN-row units (max over cores, >=1 unit)
    upt = 128 // ALIGN                                # units per tile
    U = np.maximum(1, (cnts.max(axis=0) + ALIGN - 1) // ALIGN)   # [wpc, ng]

    # stream layout: (sw, g) chunks of back-to-back runs, chunk padded to
    # whole tiles; gather ops of <= max_op rows; per-op window tile spans
    ops = []
    run_u0 = np.zeros((wpc, ng), np.int64)
    upos = 0
    mt = cfg.max_op // 128
    for sw in range(cfg.nsw):
        for g in range(ng):
            chunk_u0 = upos
            bounds = []                       # (j, unit_start, unit_end)
            for jl in range(cfg.bw):
                j = sw * cfg.bw + jl
                run_u0[j, g] = upos
                bounds.append((j, upos, upos + int(U[j, g])))
                upos += int(U[j, g])
            upos = (upos + upt - 1) // upt * upt
            t0 = chunk_u0 // upt
            ct = (upos - chunk_u0) // upt
            o = 0
            while o < ct:
                nt_op = min(mt, ct - o)
                ot0, ot1 = t0 + o, t0 + o + nt_op    # op tile range
                wins = []
                for (j, a, b) in bounds:
                    ta, tb = a // upt, (b + upt - 1) // upt
                    s, e = max(ta, ot0), min(tb, ot1)
                    if s < e:
                        islast = (g == ng - 1 and e == tb)
                        wins.append((j, s - ot0, e - s, islast))
                ops.append((ot0, nt_op, g, wins))
                o += nt_op
    ntiles = upos // upt
    lay = Layout(ops=ops, ntiles=ntiles)
    rows = ntiles * 128

    # per-core idx/slot tables in the unified layout
    idx_ws, slot_tabs = [], []
    invd_arr = np.zeros((NCORE, 128, wpc), np.float32)
    rer_arr = np.zeros((NCORE, 128, wpc), np.float32)
    for c in range(NCORE):
        s_c, d_c, g_c, j_c = es[c], ed[c], eg[c], ej[c]
        key = (j_c * ng + g_c)
        order = np.argsort(key * (1 << 17) + s_c, kind="stable")
        s_c, d_c, g_c, key = (a[order] for a in (s_c, d_c, g_c, key))
        cnt = np.bincount(key, minlength=wpc * ng)
        start = np.concatenate([[0], np.cumsum(cnt)[:-1]])
        idx_arr = np.zeros(rows, np.int64)
        slot_arr = np.full(rows, PAD_SLOT, np.float32)
        for j in range(wpc):
            ph = 128.0 * (j % cfg.bw)
            for g in range(ng):
                k = j * ng + g
                n = int(cnt[k])
                p0 = int(run_u0[j, g]) * ALIGN
                if n:
                    sl = slice(start[k], start[k] + n)
                    idx_arr[p0:p0 + n] = s_c[sl] - g_c[sl] * cfg.gs
                    slot_arr[p0:p0 + n] = ph + (d_c[sl] & 127)
                    idx_arr[p0 + n:p0 + int(U[j, g]) * ALIGN] = idx_arr[p0 + n - 1]
        idx_ws.append(np.ascontiguousarray(wrap_idx(idx_arr)))
        st = slot_arr.reshape(-1, 128).T
        slot_tabs.append(np.ascontiguousarray(st.astype(np.float16)))

        lo = c * npc
        node = lo + (perm[c][:, None] * 128 + np.arange(128)[None, :])
        valid = node < N
        dv = np.zeros((wpc, 128), np.float32)
        dv[valid] = deg_full[node[valid]]
        invd_arr[c] = (1.0 / np.maximum(dv, 1.0)).T
        rv = np.zeros((wpc, 128), np.float32)
        rv[valid] = rer[node[valid]]
        rer_arr[c] = rv.T

    xT_own = np.zeros((NCORE, 2, 128, cfg.npc), ml_dtypes.bfloat16)
    for c in range(NCORE):
        lo = c * npc
        cols = (lo + perm[c][:, None] * 128 + np.arange(128)[None, :]).ravel()
        np.clip(cols, 0, cfg.nx - 1, out=cols)
        xT_own[c, 0] = xT_bf[0:128, cols]
        xT_own[c, 1] = xT_bf[128:256, cols]
    return x_gt, idx_ws, slot_tabs, invd_arr, rer_arr, xT_own, perm, lay


def build(cfg: Cfg, lay: Layout):
    nc = bass.Bass("TRN2", target_bir_lowering=False, debug=False,
                   num_devices=NCORE, dynamic_dma_scratch_size=32768,
                   num_swdge_queues=4)
    D, H = cfg.d_in, cfg.d_h
    wpc, ntiles = cfg.wpc, lay.ntiles
    GDT = FP8 if USE_FP8 else BF16
    xrows = nc.dram_tensor("xrows", [cfg.nx, D], GDT, kind="ExternalInput")
    idx = nc.dram_tensor("idx", [128, ntiles * 8], I16, kind="ExternalInput")
    slot = nc.dram_tensor("slot", [128, ntiles], F16, kind="ExternalInput")
    invd = nc.dram_tensor("invd", [128, wpc], F32, kind="ExternalInput")
    rer = nc.dram_tensor("rer", [128, wpc], F32, kind="ExternalInput")
    xto = nc.dram_tensor("xto", [2, 128, cfg.npc], BF16, kind="ExternalInput")
    wl = nc.dram_tensor("wl", [2, 128, H], BF16, kind="ExternalInput")
    wrx = nc.dram_tensor("wrx", [2, 128, H + 1], BF16, kind="ExternalInput")
    blc = nc.dram_tensor("blc", [1, H + 1], F32, kind="ExternalInput")
    blb = nc.dram_tensor("blb", [128, H], F32, kind="ExternalInput")
    wscb = nc.dram_tensor("wscb", [128, H], F32, kind="ExternalInput")
    iota7 = nc.dram_tensor("iota7", [128, cfg.bw * 128], F16,
                           kind="ExternalInput")
    out = nc.dram_tensor("out", [128, wpc], F32, kind="ExternalOutput")

    with tile.TileContext(nc) as tc:
        with (
            tc.tile_pool(name="persist", bufs=1) as pp,
            tc.tile_pool(name="gpool", bufs=8) as gpool,
            tc.tile_pool(name="mpool", bufs=8) as mpool,
            tc.tile_pool(name="wsb", bufs=4) as wsb,
            tc.tile_pool(name="apsum", bufs=(cfg.bw + 1) // 2, space="PSUM") as apsum,
            tc.tile_pool(name="tpsum", bufs=2, space="PSUM") as tpsum,
            tc.tile_pool(name="hpsum", bufs=2, space="PSUM") as hpsum,
        ):
            # ---- persistent loads -------------------------------------
            from concourse import library_config
            nc.gpsimd.load_library(library_config.mlp)
            # first gathers need idx[0..] + slot + iota: load those first,
            # the big xto tensors last (not needed until the first epilogue)
            idx_t = pp.tile([128, ntiles * 8], I16)
            cols = ntiles * 8
            step = ((cols + cfg.nsw - 1) // cfg.nsw + 7) // 8 * 8
            nc.sync.dma_start(out=idx_t[:, 0:step], in_=idx[:, 0:step])
            slot_t = pp.tile([128, ntiles], F16)
            nc.sync.dma_start(out=slot_t[:], in_=slot[:])
            iota_t = pp.tile([128, cfg.bw * 128], F16)
            nc.sync.dma_start(out=iota_t[:], in_=iota7[:])
            for s in range(step, cols, step):
                e = min(s + step, cols)
                nc.sync.dma_start(out=idx_t[:, s:e], in_=idx[:, s:e])
            invd_t = pp.tile([128, wpc], F32)
            nc.sync.dma_start(out=invd_t[:], in_=invd[:])
            rer_t = pp.tile([128, wpc], F32)
            nc.sync.dma_start(out=rer_t[:], in_=rer[:])
            wscb_t = pp.tile([128, H], F32)
            nc.sync.dma_start(out=wscb_t[:], in_=wscb[:])
            xto_t = []
            for h in range(2):
                t = pp.tile([128, cfg.npc], BF16, tag=f"xto{h}")
                nc.sync.dma_start(out=t[:], in_=xto[h])
                xto_t.append(t)
            wl_t = []
            wrx_t = []
            for h in range(2):
                t = pp.tile([128, H], BF16, tag=f"wl{h}")
                nc.sync.dma_start(out=t[:], in_=wl[h])
                wl_t.append(t)
                t2 = pp.tile([128, H + 1], BF16, tag=f"wrx{h}")
                nc.sync.dma_start(out=t2[:], in_=wrx[h])
                wrx_t.append(t2)
            blc_t = pp.tile([1, H + 1], F32)
            nc.sync.dma_start(out=blc_t[:], in_=blc[:])
            blb_t = pp.tile([128, H], F32)
            nc.sync.dma_start(out=blb_t[:], in_=blb[:])
            ones_row = pp.tile([1, 128], F32)
            nc.vector.memset(ones_row[:], 1.0)
            out_t = pp.tile([128, wpc], F32)

            ident = pp.tile([128, 128], BF16)
            from concourse.masks import make_identity
            make_identity(nc, ident[:])

            kregs = {}
            for (_, nt, _, _) in lay.ops:
                sz = nt * 128
                if sz not in kregs:
                    kregs[sz] = nc.gpsimd.to_reg(sz)

            def epilogue(j, acc):
                aggr = wsb.tile([128, D], BF16, tag="aggr")
                nc.vector.tensor_tensor(
                    out=aggr[:], in0=acc,
                    in1=invd_t[:, j:j + 1].to_broadcast([128, D]), op=AOP.mult)
                ph = hpsum.tile([128, H + 1], F32, tag="ph")
                for h in range(2):
                    nc.tensor.matmul(
                        ph[:, 0:H + 1],
                        lhsT=xto_t[h][:, j * 128:(j + 1) * 128],
                        rhs=wrx_t[h][:], start=(h == 0), stop=False)
                if USE_BLC:
                    nc.tensor.matmul(ph[:, 0:H + 1], lhsT=ones_row[:],
                                     rhs=blc_t[:], start=False, stop=False)
                for h in range(2):
                    pt = tpsum.tile([128, 128], BF16, tag="pt")
                    nc.tensor.transpose(out=pt[:],
                                        in_=aggr[:, h * 128:(h + 1) * 128],
                                        identity=ident[:])
                    aggrT = wsb.tile([128, 128], BF16, tag=f"aggrT{h}")
                    if USE_ACTCOPY:
                        nc.scalar.activation(out=aggrT[:], in_=pt[:],
                                             func=ACT.Copy)
                    else:
                        nc.vector.tensor_copy(out=aggrT[:], in_=pt[:])
                    nc.tensor.matmul(ph[:, 0:H], lhsT=aggrT[:], rhs=wl_t[h][:],
                                     start=False, stop=(h == 1))
                if USE_BLC:
                    hin = ph[:, 0:H]
                else:
                    hpre = wsb.tile([128, H], F32, tag="hpre")
                    nc.vector.tensor_add(out=hpre[:], in0=ph[:, 0:H],
                                         in1=blb_t[:])
                    hin = hpre[:]
                hrelu = wsb.tile([128, H], F32, tag="hrelu")
                nc.scalar.activation(out=hrelu[:], in_=hin, func=ACT.Relu)
                hw = wsb.tile([128, H], F32, tag="hw")
                if USE_TTR:
                    nc.vector.tensor_tensor_reduce(
                        out=hw[:], in0=hrelu[:], in1=wscb_t[:], scale=1.0,
                        scalar=ph[:, H:H + 1], op0=AOP.mult, op1=AOP.add,
                        accum_out=out_t[:, j:j + 1])
                else:
                    nc.vector.tensor_tensor(out=hw[:], in0=hrelu[:],
                                            in1=wscb_t[:], op=AOP.mult)
                    gdot = wsb.tile([128, 1], F32, tag="gdot")
                    nc.vector.reduce_sum(out=gdot[:], in_=hw[:],
                                         axis=mybir.AxisListType.X)
                    nc.vector.tensor_add(out=out_t[:, j:j + 1], in0=gdot[:],
                                         in1=ph[:, H:H + 1])

            # ---- main loop --------------------------------------------
            accs = {}
            started = set()
            for opi, (t0, nt, g, wins) in enumerate(lay.ops):
                sz = nt * 128
                gbf = gpool.tile([128, cfg.max_op // 128, D], GDT, tag="gb")
                gb = gbf[:, 0:nt, :]
                nc.gpsimd.dma_gather(
                    out_ap=gb[:], in_ap=xrows[g * cfg.gs:(g + 1) * cfg.gs, :],
                    idxs_ap=idx_t[:, t0 * 8:t0 * 8 + sz // 16],
                    num_idxs=sz, num_idxs_reg=kregs[sz],
                    elem_size=D, queue_num=opi % 4)
                for (j, tloc, ntw, islast) in wins:
                    jl = j % cfg.bw
                    mkf = mpool.tile([128, cfg.max_op // 128, 128], GDT,
                                     tag="mk")
                    mk = mkf[:, 0:ntw, :]
                    nc.vector.tensor_tensor(
                        out=mk[:],
                        in0=slot_t[:, t0 + tloc:t0 + tloc + ntw].unsqueeze(2)
                            .to_broadcast([128, ntw, 128]),
                        in1=iota_t[:, jl * 128:(jl + 1) * 128].unsqueeze(1)
                            .to_broadcast([128, ntw, 128]),
                        op=AOP.is_equal)
                    sw = j // cfg.bw
                    pkey = (sw, jl // 2)
                    sub = jl % 2
                    if pkey not in accs:
                        accs[pkey] = apsum.tile([128, 2 * D], F32, tag="acc",
                                                name=f"accp{pkey[1]}")
                    acc = accs[pkey][:, sub * D:(sub + 1) * D]
                    st = pkey not in started
                    started.add(pkey)
                    i = 0
                    while i < ntw:
                        pair = USE_FP8 and i + 1 < ntw
                        lastm = (i + (2 if pair else 1) >= ntw) and islast
                        if pair:
                            nc.tensor.matmul(
                                acc, lhsT=mk[:, i:i + 2, :],
                                rhs=gb[:, tloc + i:tloc + i + 2, :],
                                start=st and i == 0, stop=lastm,
                                perf_mode=mybir.MatmulPerfMode.DoubleRow)
                            i += 2
                        else:
                            nc.tensor.matmul(
                                acc, lhsT=mk[:, i, :], rhs=gb[:, tloc + i, :],
                                start=st and i == 0, stop=lastm)
                            i += 1
                    if islast:
                        epilogue(j, acc)

            nc.vector.tensor_add(out=out_t[:], in0=out_t[:], in1=rer_t[:])
            nc.sync.dma_start(out=out[:], in_=out_t[:])

    return finish(nc)


def kernel_impl(x, edge_index, reranker_scores, W_l, b_l, W_r, W_res, b_res,
                w_score, b_score, alpha, trace=False):
    N = int(x.shape[0])
    cfg = Cfg(nsw=14, bw=7, nx=100096, gs=25024)
    assert cfg.npc * NCORE >= N

    (x_gt, idx_ws, slot_tabs, invd_arr, rer_arr, xT_own, perm,
     lay) = preprocess(x, edge_index, reranker_scores, cfg)

    # host-folded small-weight math
    W_l = np.asarray(W_l, np.float64)
    W_r = np.asarray(W_r, np.float64)
    W_res = np.asarray(W_res, np.float64)
    w_score = np.asarray(w_score, np.float64)
    a = float(1.0 / (1.0 + np.exp(-float(np.asarray(alpha)))))
    oma = 1.0 - a
    u = W_res.T @ w_score                      # [256]
    cterm = float(np.asarray(b_res, np.float64) @ w_score
                  + float(np.asarray(b_score)))
    wrx_host = np.zeros((2, 128, cfg.d_h + 1), np.float32)
    wl_host = np.zeros((2, 128, cfg.d_h), np.float32)
    for h in range(2):
        wrx_host[h, :, 0:cfg.d_h] = W_r.T[h * 128:(h + 1) * 128, :]
        wrx_host[h, :, cfg.d_h] = oma * u[h * 128:(h + 1) * 128]
        wl_host[h] = W_l.T[h * 128:(h + 1) * 128, :]
    blc_host = np.zeros((1, cfg.d_h + 1), np.float32)
    blc_host[0, 0:cfg.d_h] = np.asarray(b_l, np.float32)
    blc_host[0, cfg.d_h] = oma * cterm
    band = np.arange(cfg.bw * 128, dtype=np.float32).astype(np.float16)
    iota_host = np.ascontiguousarray(
        np.broadcast_to(band, (128, cfg.bw * 128)))

    common = {
        "xrows": x_gt,
        "wl": wl_host.astype(ml_dtypes.bfloat16),
        "wrx": wrx_host.astype(ml_dtypes.bfloat16),
        "blc": blc_host,
        "blb": np.ascontiguousarray(np.broadcast_to(
            np.asarray(b_l, np.float32), (128, cfg.d_h))),
        "wscb": np.ascontiguousarray(np.broadcast_to(
            (oma * w_score).astype(np.float32), (128, cfg.d_h))),
        "iota7": iota_host,
    }
    rer_const = 0.0 if USE_BLC else oma * cterm
    nc = build(cfg, lay)
    in_maps = []
    for c_i in range(NCORE):
        im = dict(common)
        im["idx"] = idx_ws[c_i]
        im["slot"] = slot_tabs[c_i]
        im["invd"] = np.ascontiguousarray(invd_arr[c_i])
        im["rer"] = np.ascontiguousarray(
            (rer_arr[c_i] * a + rer_const).astype(np.float32))
        im["xto"] = np.ascontiguousarray(xT_own[c_i])
        in_maps.append(im)

    res = run_bass_kernel_spmd(nc, in_maps, core_ids=list(range(NCORE)),
                               trace=trace)
    full = np.zeros(N, np.float32)
    for c_i in range(NCORE):
        oc = np.asarray(res.results[c_i]["out"], np.float32)  # [128, wpc]
        lo = c_i * cfg.npc
        node = lo + (perm[c_i][:, None] * 128 + np.arange(128)[None, :])
        valid = node < N
        full[node[valid]] = oc.T[valid]
    return (full, res) if trace else full


def kernel(**inputs):
    out = kernel_impl(
        np.asarray(inputs["x"]),
        np.asarray(inputs["edge_index"]),
        np.asarray(inputs["reranker_scores"]),
        np.asarray(inputs["W_l"]),
        np.asarray(inputs["b_l"]),
        np.asarray(inputs["W_r"]),
        np.asarray(inputs["W_res"]),
        np.asarray(inputs["b_res"]),
        np.asarray(inputs["w_score"]),
        np.asarray(inputs["b_score"]),
        np.asarray(inputs["alpha"]),
    )
    return out.astype(np.float32)
